# revision 6
# baseline (speedup 1.0000x reference)
"""Trainium2 Bass kernel for nn_ContrastiveLoss (stacked cross-attention t2i).

Strategy (8 NeuronCores, caption-sharded; wire-traffic optimized):
  - The axon tunnel to the devices is the bottleneck (~46 MB/s), so inputs
    are minimized: im is shipped SHARDED (1/8 per core, fp16) and
    all-gathered on-device over NeuronLink; s is shipped per-core (its own
    16 captions, fp16); Gram matrices, eye/noteye and the broadcast wfac
    are built on-device.
  - Each core owns 16 of the 128 captions and all 128 images.  Per batch of
    3 images x 16 captions, compute A = im @ s^T on the PE (fp16 in, f32
    accum), the two softmaxes (word softmax normalized; region softmax's
    normalizer cancels inside the cosine, so only exp(9*a1 - 9) is needed
    -- the e^-9 scaling keeps everything fp16-safe), the cosine
    numerator/denominator via PE column sums, staged into [128, 800] tiles.
  - One finalization pass turns staged tiles into the [128, 16] score block.
  - AllGather score blocks -> every core holds scores [128, 128]; the hinge
    margin loss (max violation) is computed on-device; host reads core 0's
    scalar.
  - The jitted shard_map dispatch is built once and cached; per-call work is
    host prep (~0.1 s), ~24 MB of input transfer, and the device run.

Math note: with E2 = exp(lam * a1 - lam) (unnormalized region attention),
  cos = (sum_r E2*A) / (cap_n * sqrt(E2^T G E2)) exactly, because both the
region-softmax normalizer and the e^-lam scaling cancel between numerator
and |weighted context|.
"""

import numpy as np

import concourse.bass as bass
import concourse.tile as tile
from concourse import mybir
from concourse.vector_clock import ScopedClock

# ---------------------------------------------------------------------------
# Workaround for this toolchain: walrus rejects instructions carrying more
# than one semaphore wait.  Split extra waits onto standalone EventSemaphore
# instructions (the same thing wait_ge emits) just before the offender.
# ---------------------------------------------------------------------------
_PATCHED = False


def _install_patches():
    global _PATCHED
    if _PATCHED:
        return
    _PATCHED = True

    def _drain_and_barrier(self, tick_clock, wait_clock):
        nc = self.nc
        drain_inst = nc.sync.drain()
        wait_clock.add_sem_waits(
            drain_inst.ins, ScopedClock({None: tick_clock.global_clock})
        )
        waits = list(drain_inst.ins.sync_info.on_wait)
        if len(waits) > 1:
            drain_inst.ins.sync_info.on_wait = waits[:1]
            for w in waits[1:]:
                extra = nc.sync.drain()
                extra.ins.sync_info = mybir.SyncInfo(on_wait=[w], on_update=[])
        nc.all_engine_barrier()
        popped = nc._tile_sem_poison_stack.pop()
        assert popped is self._sem_poison
        nc.clear_and_free_semaphores(list(self.sems.allocated().values()))
        nc.all_engine_barrier()

    tile.TileContext._drain_and_barrier = _drain_and_barrier

    import concourse.bass_utils as bass_utils
    import concourse.bass2jax as bass2jax
    import orjson

    _orig_compile = bass_utils.compile_bir_kernel

    def _split_waits_in_bir(bir_json: bytes) -> bytes:
        m = orjson.loads(bir_json)
        for fn in m.get("functions", []):
            for blk in fn.get("blocks", []):
                insts = blk.get("instructions", [])
                new_insts = []
                for ins in insts:
                    si = ins.get("sync_info")
                    waits = (si or {}).get("on_wait") or []
                    if len(waits) > 1:
                        for k, w in enumerate(waits[:-1]):
                            new_insts.append(
                                {
                                    "name": f"{ins['name']}_wsplit{k}",
                                    "opcode": "EventSemaphore",
                                    "engine": ins["engine"],
                                    "ins": [],
                                    "outs": [],
                                    "debug": ins.get("debug"),
                                    "sync_info": {"on_update": [], "on_wait": [w]},
                                }
                            )
                        si["on_wait"] = waits[-1:]
                    new_insts.append(ins)
                blk["instructions"] = new_insts
        return orjson.dumps(m)

    def _patched_compile(bir_json, tmpdir, neff_name="file.neff"):
        return _orig_compile(_split_waits_in_bir(bir_json), tmpdir, neff_name)

    bass_utils.compile_bir_kernel = _patched_compile
    bass2jax.compile_bir_kernel = _patched_compile


# ---------------------------------------------------------------------------
# Problem constants (hardcoded per the task contract).
# ---------------------------------------------------------------------------
B = 128           # images == captions
LI = 36           # image regions
LW = 50           # padded caption words
D = 1024          # feature dim
NC = 8            # cores
CAP = B // NC     # captions per core (16)
WF = CAP * LW     # free width of the batched tiles (800)
IMG_GRP = 3       # images per batch
GP = IMG_GRP * LI  # partitions per full batch (108)
NB = (B + IMG_GRP - 1) // IMG_GRP  # 43 batches (42x3 + 1x2)
SH = B * LI // NC  # im columns per shard (576)
LAM = 9.0
MARGIN = 0.2
EPS = 1e-8
MASKNEG = -30000.0

F32 = mybir.dt.float32
F32R = mybir.dt.float32r
F16 = mybir.dt.float16
I32 = mybir.dt.int32

_CACHE = {}


def _build_program():
    nc = bass.Bass("TRN2", target_bir_lowering=False, debug=False, num_devices=NC)

    # Inputs (per-core contents differ only for sTh / mneg / wfacr).
    imh = nc.dram_tensor("imh", [8, 128, SH], F16, kind="ExternalInput")
    sTh = nc.dram_tensor("sTh", [8, 128, WF], F16, kind="ExternalInput")
    mneg = nc.dram_tensor("mneg", [1, WF], F16, kind="ExternalInput")
    wfacr = nc.dram_tensor("wfacr", [1, WF], F32R, kind="ExternalInput")
    bmask = nc.dram_tensor("bmask", [GP, GP], F16, kind="ExternalInput")
    oblk = nc.dram_tensor("oblk", [GP, IMG_GRP], F16, kind="ExternalInput")
    orow = nc.dram_tensor("orow", [1, GP], F16, kind="ExternalInput")
    o128r = nc.dram_tensor("o128r", [1, 128], F32R, kind="ExternalInput")
    oc128 = nc.dram_tensor("oc128", [128, 1], F32R, kind="ExternalInput")

    loss_out = nc.dram_tensor("loss_out", [1, 2], F32, kind="ExternalOutput")
    scores_out = nc.dram_tensor("scores_out", [128, 128], F32, kind="ExternalOutput")

    with tile.TileContext(nc) as tc:
        with (
            tc.tile_pool(name="const", bufs=1) as cpool,
            tc.tile_pool(name="work", bufs=2) as work,
            tc.tile_pool(name="small", bufs=2) as small,
            tc.tile_pool(name="stage", bufs=1) as stage,
            tc.tile_pool(name="pa", bufs=2, space="PSUM") as pa,
            tc.tile_pool(name="pc", bufs=2, space="PSUM") as pc,
            tc.tile_pool(name="dram", bufs=1, space="DRAM") as dram,
        ):
            NCH = [(0, 512), (512, WF)]

            # ---- persistent consts ------------------------------------------------
            sT = cpool.tile([128, 8, WF], F16, tag="sT")
            nc.sync.dma_start(sT[:], sTh[:].transpose([1, 0, 2]))
            mnegt = cpool.tile([1, WF], F16, tag="mn")
            nc.sync.dma_start(mnegt[:], mneg[:])
            wfacrt = cpool.tile([1, WF], F32R, tag="wfr")
            nc.sync.dma_start(wfacrt[:], wfacr[:])
            bmaskt = cpool.tile([GP, GP], F16, tag="bm")
            nc.sync.dma_start(bmaskt[:], bmask[:])
            oblkt = cpool.tile([GP, IMG_GRP], F16, tag="ob")
            nc.sync.dma_start(oblkt[:], oblk[:])
            orowt = cpool.tile([1, GP], F16, tag="or")
            nc.sync.dma_start(orowt[:], orow[:])
            o128rt = cpool.tile([1, 128], F32R, tag="o128r")
            nc.sync.dma_start(o128rt[:], o128r[:])
            oc128t = cpool.tile([128, 1], F32R, tag="oc128")
            nc.sync.dma_start(oc128t[:], oc128[:])

            # eye / noteye generated on-device (saves wire bytes)
            iit = cpool.tile([128, 128], I32, tag="ii")
            nc.gpsimd.iota(iit[:], pattern=[[1, 128]], base=0, channel_multiplier=-1)
            eyet = cpool.tile([128, 128], F32, tag="eye")
            nc.vector.tensor_scalar(
                eyet[:], iit[:], scalar1=0, scalar2=None,
                op0=mybir.AluOpType.is_equal,
            )
            neyet = cpool.tile([128, 128], F32, tag="neye")
            nc.vector.tensor_scalar(
                neyet[:], iit[:], scalar1=0, scalar2=None,
                op0=mybir.AluOpType.not_equal,
            )

            lamneg = cpool.tile([128, 1], F32, tag="lamneg")
            nc.vector.memset(lamneg[:], -LAM)

            # wfac broadcast to all 128 partitions via K=1 matmul
            wf_ps = pc.tile([128, WF], F32, tag="cs")
            for n0, n1 in NCH:
                nc.tensor.matmul(
                    wf_ps[:, n0:n1], o128rt[0:1, :], wfacrt[0:1, n0:n1],
                    start=True, stop=True,
                )
            wfact = cpool.tile([128, WF], F32, tag="wf")
            nc.scalar.copy(wfact[:], wf_ps[:])

            # ---- all-gather im shards over NeuronLink -----------------------------
            agi = dram.tile([8, 128, SH], F16)
            nc.sync.dma_start(agi[:], imh[:])
            ago = dram.tile([NC, 8, 128, SH], F16, addr_space="Shared")
            nc.gpsimd.collective_compute(
                "AllGather",
                mybir.AluOpType.bypass,
                replica_groups=[list(range(NC))],
                ins=[agi.opt()],
                outs=[ago.opt()],
            )
            imf = cpool.tile([128, 8, B * LI], F16, tag="imf")
            for sh in range(NC):
                nc.sync.dma_start(
                    imf[:, :, sh * SH : (sh + 1) * SH],
                    ago[sh].transpose([1, 0, 2]),
                )

            # ---- staging tiles ----------------------------------------------------
            nst = stage.tile([128, WF], F32, tag="nst")
            wst = stage.tile([128, WF], F32, tag="wst")
            gst = stage.tile([GP, NB * GP], F16, tag="gst")

            # ---- G pass: all 43 block-diagonal Gram blocks ------------------------
            for b in range(NB):
                ng = min(IMG_GRP, B - b * IMG_GRP)
                P = ng * LI
                c0 = b * GP
                g_ps = pa.tile([GP, WF], F32, tag="AT")
                for c in range(8):
                    nc.tensor.matmul(
                        g_ps[0:P, 0:P], imf[:, c, c0 : c0 + P], imf[:, c, c0 : c0 + P],
                        start=(c == 0), stop=(c == 7),
                    )
                nc.vector.tensor_tensor(
                    gst[0:P, c0 : c0 + P], g_ps[0:P, 0:P], bmaskt[0:P, 0:P],
                    op=mybir.AluOpType.mult,
                )

            # ---- main loop over image groups -------------------------------------
            for b in range(NB):
                ng = min(IMG_GRP, B - b * IMG_GRP)
                P = ng * LI
                c0 = b * GP

                # A[P, WF] = sum_c imf_c^T @ sT_c  (+ word mask row)
                a_ps = pa.tile([P, WF], F32, tag="AT")
                for n0, n1 in NCH:
                    for c in range(8):
                        nc.tensor.matmul(
                            a_ps[:, n0:n1], imf[:, c, c0 : c0 + P], sT[:, c, n0:n1],
                            start=(c == 0), stop=False,
                        )
                    nc.tensor.matmul(
                        a_ps[:, n0:n1], orowt[0:1, 0:P], mnegt[0:1, n0:n1],
                        start=False, stop=True,
                    )

                am = work.tile([P, WF], F32, tag="am")
                nc.scalar.copy(am[:], a_ps[:])
                mx = small.tile([P, CAP], F32, tag="mx")
                nc.vector.tensor_reduce(
                    mx[:], a_ps[:].rearrange("p (c w) -> p c w", c=CAP, w=LW),
                    axis=mybir.AxisListType.X, op=mybir.AluOpType.max,
                )
                sub = work.tile([P, WF], F32, tag="sub")
                nc.gpsimd.tensor_tensor(
                    sub[:].rearrange("p (c w) -> p c w", c=CAP, w=LW),
                    am[:].rearrange("p (c w) -> p c w", c=CAP, w=LW),
                    mx[:].unsqueeze(2).broadcast_to([P, CAP, LW]),
                    op=mybir.AluOpType.subtract,
                )
                e = work.tile([P, WF], F32, tag="e")
                nc.scalar.activation(e[:], sub[:], mybir.ActivationFunctionType.Exp)

                z = small.tile([P, CAP], F32, tag="z")
                nc.vector.tensor_reduce(
                    z[:], e[:].rearrange("p (c w) -> p c w", c=CAP, w=LW),
                    axis=mybir.AxisListType.X, op=mybir.AluOpType.add,
                )
                rz = small.tile([P, CAP], F32, tag="rz")
                nc.vector.reciprocal(rz[:], z[:])

                m = work.tile([P, WF], F32, tag="m")
                nc.vector.tensor_tensor(
                    m[:].rearrange("p (c w) -> p c w", c=CAP, w=LW),
                    e[:].rearrange("p (c w) -> p c w", c=CAP, w=LW),
                    rz[:].unsqueeze(2).broadcast_to([P, CAP, LW]),
                    op=mybir.AluOpType.mult,
                )
                # e2 = exp(lam*m - lam) in fp16; the e^-lam scaling cancels in cos
                e2 = work.tile([P, WF], F16, tag="e2")
                nc.scalar.activation(
                    e2[:], m[:], mybir.ActivationFunctionType.Exp,
                    bias=lamneg[0:P, :], scale=LAM,
                )

                f = work.tile([P, WF], F16, tag="f")
                nc.gpsimd.tensor_tensor(f[:], am[:], e2[:], op=mybir.AluOpType.mult)

                t_ps = pa.tile([P, WF], F32, tag="AT")
                for n0, n1 in NCH:
                    nc.tensor.matmul(
                        t_ps[:, n0:n1], gst[0:P, c0 : c0 + P], e2[:, n0:n1],
                        start=True, stop=True,
                    )

                u = work.tile([P, WF], F16, tag="u")
                nc.vector.tensor_tensor(u[:], t_ps[:], e2[:], op=mybir.AluOpType.mult)

                n_ps = pc.tile([ng, WF], F32, tag="cs")
                for n0, n1 in NCH:
                    nc.tensor.matmul(
                        n_ps[:, n0:n1], oblkt[0:P, 0:ng], f[:, n0:n1],
                        start=True, stop=True,
                    )
                w_ps = pc.tile([ng, WF], F32, tag="cs")
                for n0, n1 in NCH:
                    nc.tensor.matmul(
                        w_ps[:, n0:n1], oblkt[0:P, 0:ng], u[:, n0:n1],
                        start=True, stop=True,
                    )

                r0 = b * IMG_GRP
                nb_sb = small.tile([ng, WF], F32, tag="nb_sb")
                wb_sb = small.tile([ng, WF], F32, tag="wb_sb")
                nc.scalar.copy(nb_sb[:], n_ps[:])
                nc.scalar.copy(wb_sb[:], w_ps[:])
                nc.sync.dma_start(nst[r0 : r0 + ng, :], nb_sb[:])
                nc.sync.dma_start(wst[r0 : r0 + ng, :], wb_sb[:])

            # ---- finalize: scores block [128 images, 16 captions] ----------------
            srt = work.tile([128, WF], F32, tag="am")
            nc.scalar.sqrt(srt[:], wst[:])
            q = work.tile([128, WF], F32, tag="e")
            nc.vector.tensor_tensor(q[:], nst[:], wfact[:], op=mybir.AluOpType.mult)
            rsq = work.tile([128, WF], F32, tag="sub")
            nc.vector.reciprocal(rsq[:], srt[:])
            cosq = work.tile([128, WF], F32, tag="m")
            nc.vector.tensor_tensor(cosq[:], q[:], rsq[:], op=mybir.AluOpType.mult)
            sim = small.tile([128, CAP], F32, tag="sim")
            nc.vector.tensor_reduce(
                sim[:], cosq[:].rearrange("p (c w) -> p c w", c=CAP, w=LW),
                axis=mybir.AxisListType.X, op=mybir.AluOpType.add,
            )

            # ---- all-gather the score columns ------------------------------------
            ag_in = dram.tile([128, CAP], F32)
            ag_out = dram.tile([NC, 128, CAP], F32, addr_space="Shared")
            nc.sync.dma_start(ag_in[:], sim[:])
            nc.gpsimd.collective_compute(
                "AllGather",
                mybir.AluOpType.bypass,
                replica_groups=[list(range(NC))],
                ins=[ag_in.opt()],
                outs=[ag_out.opt()],
            )
            s_t = cpool.tile([128, NC, CAP], F32, tag="scores")
            nc.sync.dma_start(s_t[:], ag_out[:].transpose([1, 0, 2]))
            s2d = s_t[:].rearrange("p c w -> p (c w)")
            nc.sync.dma_start(scores_out[:], s2d)

            # ---- margin loss (every core computes it; core 0's is read) ----------
            junk = work.tile([128, 128], F32, tag="am")
            diag = small.tile([128, 1], F32, tag="diag")
            nc.vector.tensor_tensor(junk[:, 0:128], s2d, eyet[:], op=mybir.AluOpType.mult)
            nc.vector.tensor_reduce(
                diag[:], junk[:, 0:128], axis=mybir.AxisListType.X, op=mybir.AluOpType.add
            )
            bias = small.tile([128, 1], F32, tag="bias")
            nc.vector.tensor_scalar(
                bias[:], diag[:], scalar1=-1.0, scalar2=MARGIN,
                op0=mybir.AluOpType.mult, op1=mybir.AluOpType.add,
            )
            # cost_s = relu(S + margin - d_i), diagonal zeroed
            cs = work.tile([128, 128], F32, tag="e")
            nc.scalar.activation(
                cs[:], s2d, mybir.ActivationFunctionType.Relu, bias=bias[:], scale=1.0
            )
            cs2 = work.tile([128, 128], F32, tag="m")
            nc.vector.tensor_tensor(cs2[:], cs[:], neyet[:], op=mybir.AluOpType.mult)
            rmaxs = small.tile([128, 2], F32R, tag="rmaxs")
            nc.vector.tensor_reduce(
                rmaxs[:, 0:1], cs2[:], axis=mybir.AxisListType.X, op=mybir.AluOpType.max
            )
            # transposed scores for cost_im
            st_ps = pc.tile([128, 128], F32, tag="cs")
            nc.tensor.transpose(st_ps[:], s_t[:].rearrange("p c w -> p (c w)"), eyet[:])
            ct = work.tile([128, 128], F32, tag="u")
            nc.scalar.activation(
                ct[:], st_ps[:], mybir.ActivationFunctionType.Relu, bias=bias[:], scale=1.0
            )
            ct2 = work.tile([128, 128], F32, tag="f")
            nc.vector.tensor_tensor(ct2[:], ct[:], neyet[:], op=mybir.AluOpType.mult)
            nc.vector.tensor_reduce(
                rmaxs[:, 1:2], ct2[:], axis=mybir.AxisListType.X, op=mybir.AluOpType.max
            )
            tot_ps = pc.tile([1, 2], F32, tag="cs")
            nc.tensor.matmul(tot_ps[:], oc128t[:], rmaxs[:], start=True, stop=True)
            tot = small.tile([1, 2], F32, tag="tot")
            nc.scalar.copy(tot[:], tot_ps[:])
            nc.sync.dma_start(loss_out[:], tot[:])

    return nc


def _host_prep(im, s, s_l):
    """Build the global (concatenated-over-cores) input arrays."""
    im = np.ascontiguousarray(im, dtype=np.float32)
    s = np.ascontiguousarray(s, dtype=np.float32)
    s_l = np.asarray(s_l).astype(np.int64)

    # imh global [NC*8, 128, SH]: core c gets imT8[:, :, c*SH:(c+1)*SH]
    im16 = im.astype(np.float16)
    imh_g = np.ascontiguousarray(
        im16.reshape(NC, SH, 8, 128).transpose(0, 2, 3, 1)
    ).reshape(NC * 8, 128, SH)

    # sTh global [NC*8, 128, WF]: core c gets its own 16 captions, transposed
    s16 = s.astype(np.float16)
    sTh_g = np.ascontiguousarray(
        s16.reshape(NC, WF, 8, 128).transpose(0, 2, 3, 1)
    ).reshape(NC * 8, 128, WF)

    wmask = np.arange(LW)[None, :] < s_l[:, None]                  # [B, LW]
    capn = np.sqrt(np.einsum("bwd,bwd->bw", s, s))                 # [B, LW]
    lens = s_l.astype(np.float32)[:, None]
    mneg_g = ((~wmask) * np.float32(MASKNEG)).astype(np.float16).reshape(NC, WF)
    wfac_g = (
        (wmask / (np.maximum(capn, EPS) * lens)).astype(np.float32).reshape(NC, WF)
    )

    bm = np.zeros((GP, GP), dtype=np.float16)
    ob = np.zeros((GP, IMG_GRP), dtype=np.float16)
    for g in range(IMG_GRP):
        bm[g * LI : (g + 1) * LI, g * LI : (g + 1) * LI] = 1.0
        ob[g * LI : (g + 1) * LI, g] = 1.0
    bmask_g = np.ascontiguousarray(np.broadcast_to(bm, (NC, GP, GP))).reshape(
        NC * GP, GP
    )
    oblk_g = np.ascontiguousarray(np.broadcast_to(ob, (NC, GP, IMG_GRP))).reshape(
        NC * GP, IMG_GRP
    )
    orow_g = np.ones((NC * 1, GP), dtype=np.float16)
    o128r_g = np.ones((NC * 1, 128), dtype=np.float32)
    oc128_g = np.ones((NC * 128, 1), dtype=np.float32)

    return {
        "imh": imh_g,
        "sTh": sTh_g,
        "mneg": mneg_g,
        "wfacr": wfac_g,
        "bmask": bmask_g,
        "oblk": oblk_g,
        "orow": orow_g,
        "o128r": o128r_g,
        "oc128": oc128_g,
    }


def _get_state():
    if "state" in _CACHE:
        return _CACHE["state"]
    _install_patches()

    import jax
    from jax.sharding import Mesh, PartitionSpec, NamedSharding
    import warnings

    with warnings.catch_warnings():
        warnings.simplefilter("ignore")
        from jax.experimental.shard_map import shard_map

    from concourse.bass2jax import (
        _bass_exec_p,
        install_neuronx_cc_hook,
        partition_id_tensor,
    )

    install_neuronx_cc_hook()
    nc = _build_program()

    partition_name = nc.partition_id_tensor.name if nc.partition_id_tensor else None
    in_names, out_names, out_avals, zero_shapes = [], [], [], []
    for alloc in nc.m.functions[0].allocations:
        if not isinstance(alloc, mybir.MemoryLocationSet):
            continue
        name = alloc.memorylocations[0].name
        if alloc.kind == "ExternalInput":
            if name != partition_name:
                in_names.append(name)
        elif alloc.kind == "ExternalOutput":
            shape = tuple(alloc.tensor_shape)
            dtype = mybir.dt.np(alloc.dtype)
            out_names.append(name)
            out_avals.append(jax.core.ShapedArray(shape, dtype))
            zero_shapes.append((shape, dtype))
    n_params = len(in_names)
    n_outs = len(out_avals)
    in_names_all = in_names + out_names
    if partition_name is not None:
        in_names_all = in_names_all + [partition_name]
    donate = tuple(range(n_params, n_params + n_outs))

    def _body(*args):
        operands = list(args)
        if partition_name is not None:
            operands.append(partition_id_tensor())
        outs = _bass_exec_p.bind(
            *operands,
            out_avals=tuple(out_avals),
            in_names=tuple(in_names_all),
            out_names=tuple(out_names),
            lowering_input_output_aliases=(),
            sim_require_finite=True,
            sim_require_nnan=True,
            nc=nc,
        )
        return tuple(outs)

    devices = jax.devices()[:NC]
    assert len(devices) == NC
    mesh = Mesh(np.asarray(devices), ("core",))
    spec = NamedSharding(mesh, PartitionSpec("core"))
    sharded = jax.jit(
        shard_map(
            _body,
            mesh=mesh,
            in_specs=(PartitionSpec("core"),) * (n_params + n_outs),
            out_specs=(PartitionSpec("core"),) * n_outs,
            check_rep=False,
        ),
        donate_argnums=donate,
        keep_unused=True,
    )

    state = {
        "nc": nc,
        "sharded": sharded,
        "spec": spec,
        "in_names": in_names,
        "out_names": out_names,
        "zero_shapes": zero_shapes,
        "loss_idx": out_names.index("loss_out"),
        "scores_idx": out_names.index("scores_out"),
    }
    _CACHE["state"] = state
    return state


def run(im, s, s_l, fetch_scores=False, trace=False):
    """Returns (loss_scalar, scores[128,128] or None)."""
    import jax

    st = _get_state()
    arrays = _host_prep(im, s, s_l)
    dev_in = [jax.device_put(arrays[n], st["spec"]) for n in st["in_names"]]
    zeros = [np.zeros((NC * sh[0], *sh[1:]), dt) for sh, dt in st["zero_shapes"]]
    outs = st["sharded"](*dev_in, *zeros)
    loss_arr = outs[st["loss_idx"]]
    try:
        l0 = np.asarray(loss_arr.addressable_shards[0].data)
    except Exception:
        l0 = np.asarray(loss_arr)[0:1]
    loss = np.float32(l0[0, 0] + l0[0, 1])
    scores = None
    if fetch_scores:
        scores = np.asarray(outs[st["scores_idx"]])[0:128]
    return loss, scores


def kernel(im, s, s_l):
    loss, _ = run(im, s, s_l)
    return np.array(loss, dtype=np.float32)


# revision 9
# speedup vs baseline: 1.1596x; 1.1596x over previous
"""Trainium2 Bass kernel for nn_ContrastiveLoss (stacked cross-attention t2i).

Strategy (8 NeuronCores, caption-sharded; wire-traffic optimized):
  - The axon tunnel to the devices is the bottleneck (~46 MB/s), so inputs
    are minimized: im is shipped SHARDED (1/8 per core, fp16) and
    all-gathered on-device over NeuronLink; s is shipped per-core (its own
    16 captions, fp16); Gram matrices, eye/noteye and the broadcast wfac
    are built on-device.
  - Each core owns 16 of the 128 captions and all 128 images.  Per batch of
    3 images x 16 captions, compute A = im @ s^T on the PE (fp16 in, f32
    accum), the two softmaxes (word softmax normalized; region softmax's
    normalizer cancels inside the cosine, so only exp(9*a1 - 9) is needed
    -- the e^-9 scaling keeps everything fp16-safe), the cosine
    numerator/denominator via PE column sums, staged into [128, 800] tiles.
  - One finalization pass turns staged tiles into the [128, 16] score block.
  - AllGather score blocks -> every core holds scores [128, 128]; the hinge
    margin loss (max violation) is computed on-device; host reads core 0's
    scalar.
  - The jitted shard_map dispatch is built once and cached; per-call work is
    host prep (~0.1 s), ~24 MB of input transfer, and the device run.

Math note: with E2 = exp(lam * a1 - lam) (unnormalized region attention),
  cos = (sum_r E2*A) / (cap_n * sqrt(E2^T G E2)) exactly, because both the
region-softmax normalizer and the e^-lam scaling cancel between numerator
and |weighted context|.
"""

import numpy as np

import concourse.bass as bass
import concourse.tile as tile
from concourse import mybir
from concourse.vector_clock import ScopedClock

# ---------------------------------------------------------------------------
# Workaround for this toolchain: walrus rejects instructions carrying more
# than one semaphore wait.  Split extra waits onto standalone EventSemaphore
# instructions (the same thing wait_ge emits) just before the offender.
# ---------------------------------------------------------------------------
_PATCHED = False


def _install_patches():
    global _PATCHED
    if _PATCHED:
        return
    _PATCHED = True

    def _drain_and_barrier(self, tick_clock, wait_clock):
        nc = self.nc
        drain_inst = nc.sync.drain()
        wait_clock.add_sem_waits(
            drain_inst.ins, ScopedClock({None: tick_clock.global_clock})
        )
        waits = list(drain_inst.ins.sync_info.on_wait)
        if len(waits) > 1:
            drain_inst.ins.sync_info.on_wait = waits[:1]
            for w in waits[1:]:
                extra = nc.sync.drain()
                extra.ins.sync_info = mybir.SyncInfo(on_wait=[w], on_update=[])
        nc.all_engine_barrier()
        popped = nc._tile_sem_poison_stack.pop()
        assert popped is self._sem_poison
        nc.clear_and_free_semaphores(list(self.sems.allocated().values()))
        nc.all_engine_barrier()

    tile.TileContext._drain_and_barrier = _drain_and_barrier

    import concourse.bass_utils as bass_utils
    import concourse.bass2jax as bass2jax
    import orjson

    _orig_compile = bass_utils.compile_bir_kernel

    def _split_waits_in_bir(bir_json: bytes) -> bytes:
        m = orjson.loads(bir_json)
        for fn in m.get("functions", []):
            for blk in fn.get("blocks", []):
                insts = blk.get("instructions", [])
                new_insts = []
                for ins in insts:
                    si = ins.get("sync_info")
                    waits = (si or {}).get("on_wait") or []
                    if len(waits) > 1:
                        for k, w in enumerate(waits[:-1]):
                            new_insts.append(
                                {
                                    "name": f"{ins['name']}_wsplit{k}",
                                    "opcode": "EventSemaphore",
                                    "engine": ins["engine"],
                                    "ins": [],
                                    "outs": [],
                                    "debug": ins.get("debug"),
                                    "sync_info": {"on_update": [], "on_wait": [w]},
                                }
                            )
                        si["on_wait"] = waits[-1:]
                    new_insts.append(ins)
                blk["instructions"] = new_insts
        return orjson.dumps(m)

    def _patched_compile(bir_json, tmpdir, neff_name="file.neff"):
        return _orig_compile(_split_waits_in_bir(bir_json), tmpdir, neff_name)

    bass_utils.compile_bir_kernel = _patched_compile
    bass2jax.compile_bir_kernel = _patched_compile


# ---------------------------------------------------------------------------
# Problem constants (hardcoded per the task contract).
# ---------------------------------------------------------------------------
B = 128           # images == captions
LI = 36           # image regions
LW = 50           # padded caption words
D = 1024          # feature dim
NC = 8            # cores
CAP = B // NC     # captions per core (16)
WF = CAP * LW     # free width of the batched tiles (800)
IMG_GRP = 3       # images per batch
GP = IMG_GRP * LI  # partitions per full batch (108)
NB = (B + IMG_GRP - 1) // IMG_GRP  # 43 batches (42x3 + 1x2)
SH = B * LI // NC  # im columns per shard (576)
LAM = 9.0
MARGIN = 0.2
EPS = 1e-8
MASKNEG = -30000.0

F32 = mybir.dt.float32
F32R = mybir.dt.float32r
F16 = mybir.dt.float16
I32 = mybir.dt.int32

_CACHE = {}


def _build_program():
    nc = bass.Bass("TRN2", target_bir_lowering=False, debug=False, num_devices=NC)

    # Inputs (per-core contents differ only for sTh / mneg / wfacr).
    imh = nc.dram_tensor("imh", [8, 128, SH], F16, kind="ExternalInput")
    sTh = nc.dram_tensor("sTh", [8, 128, WF], F16, kind="ExternalInput")
    mneg = nc.dram_tensor("mneg", [1, WF], F16, kind="ExternalInput")
    wfacr = nc.dram_tensor("wfacr", [1, WF], F32R, kind="ExternalInput")
    bmask = nc.dram_tensor("bmask", [GP, GP], F16, kind="ExternalInput")
    oblk = nc.dram_tensor("oblk", [GP, IMG_GRP], F16, kind="ExternalInput")
    orow = nc.dram_tensor("orow", [1, GP], F16, kind="ExternalInput")
    o128r = nc.dram_tensor("o128r", [1, 128], F32R, kind="ExternalInput")
    oc128 = nc.dram_tensor("oc128", [128, 1], F32R, kind="ExternalInput")

    loss_out = nc.dram_tensor("loss_out", [1, 2], F32, kind="ExternalOutput")
    scores_out = nc.dram_tensor("scores_out", [128, 128], F32, kind="ExternalOutput")

    with tile.TileContext(nc) as tc:
        with (
            tc.tile_pool(name="const", bufs=1) as cpool,
            tc.tile_pool(name="work", bufs=2) as work,
            tc.tile_pool(name="small", bufs=2) as small,
            tc.tile_pool(name="stage", bufs=1) as stage,
            tc.tile_pool(name="pa", bufs=2, space="PSUM") as pa,
            tc.tile_pool(name="pc", bufs=2, space="PSUM") as pc,
            tc.tile_pool(name="dram", bufs=1, space="DRAM") as dram,
        ):
            NCH = [(0, 512), (512, WF)]

            # ---- persistent consts ------------------------------------------------
            sT = cpool.tile([128, 8, WF], F16, tag="sT")
            nc.sync.dma_start(sT[:], sTh[:].transpose([1, 0, 2]))
            mnegt = cpool.tile([1, WF], F16, tag="mn")
            nc.sync.dma_start(mnegt[:], mneg[:])
            wfacrt = cpool.tile([1, WF], F32R, tag="wfr")
            nc.sync.dma_start(wfacrt[:], wfacr[:])
            bmaskt = cpool.tile([GP, GP], F16, tag="bm")
            nc.sync.dma_start(bmaskt[:], bmask[:])
            oblkt = cpool.tile([GP, IMG_GRP], F16, tag="ob")
            nc.sync.dma_start(oblkt[:], oblk[:])
            orowt = cpool.tile([1, GP], F16, tag="or")
            nc.sync.dma_start(orowt[:], orow[:])
            o128rt = cpool.tile([1, 128], F32R, tag="o128r")
            nc.sync.dma_start(o128rt[:], o128r[:])
            oc128t = cpool.tile([128, 1], F32R, tag="oc128")
            nc.sync.dma_start(oc128t[:], oc128[:])

            # eye / noteye generated on-device (saves wire bytes)
            iit = cpool.tile([128, 128], I32, tag="ii")
            nc.gpsimd.iota(iit[:], pattern=[[1, 128]], base=0, channel_multiplier=-1)
            eyet = cpool.tile([128, 128], F32, tag="eye")
            nc.vector.tensor_scalar(
                eyet[:], iit[:], scalar1=0, scalar2=None,
                op0=mybir.AluOpType.is_equal,
            )
            neyet = cpool.tile([128, 128], F32, tag="neye")
            nc.vector.tensor_scalar(
                neyet[:], iit[:], scalar1=0, scalar2=None,
                op0=mybir.AluOpType.not_equal,
            )

            lamneg = cpool.tile([128, 1], F32, tag="lamneg")
            nc.vector.memset(lamneg[:], -LAM)

            # wfac broadcast to all 128 partitions via K=1 matmul
            wf_ps = pc.tile([128, WF], F32, tag="cs")
            for n0, n1 in NCH:
                nc.tensor.matmul(
                    wf_ps[:, n0:n1], o128rt[0:1, :], wfacrt[0:1, n0:n1],
                    start=True, stop=True,
                )
            wfact = cpool.tile([128, WF], F32, tag="wf")
            nc.scalar.copy(wfact[:], wf_ps[:])

            # ---- all-gather im shards over NeuronLink -----------------------------
            agi = dram.tile([8, 128, SH], F16)
            nc.sync.dma_start(agi[:], imh[:])
            ago = dram.tile([NC, 8, 128, SH], F16, addr_space="Shared")
            nc.gpsimd.collective_compute(
                "AllGather",
                mybir.AluOpType.bypass,
                replica_groups=[list(range(NC))],
                ins=[agi.opt()],
                outs=[ago.opt()],
            )
            imf = cpool.tile([128, 8, B * LI], F16, tag="imf")
            for sh in range(NC):
                nc.sync.dma_start(
                    imf[:, :, sh * SH : (sh + 1) * SH],
                    ago[sh].transpose([1, 0, 2]),
                )

            # ---- staging tiles ----------------------------------------------------
            nst = stage.tile([128, WF], F32, tag="nst")
            wst = stage.tile([128, WF], F32, tag="wst")
            gst = stage.tile([GP, NB * GP], F16, tag="gst")

            # ---- G pass: all 43 block-diagonal Gram blocks ------------------------
            for b in range(NB):
                ng = min(IMG_GRP, B - b * IMG_GRP)
                P = ng * LI
                c0 = b * GP
                g_ps = pa.tile([GP, WF], F32, tag="AT")
                for c in range(8):
                    nc.tensor.matmul(
                        g_ps[0:P, 0:P], imf[:, c, c0 : c0 + P], imf[:, c, c0 : c0 + P],
                        start=(c == 0), stop=(c == 7),
                    )
                nc.vector.tensor_tensor(
                    gst[0:P, c0 : c0 + P], g_ps[0:P, 0:P], bmaskt[0:P, 0:P],
                    op=mybir.AluOpType.mult,
                )

            # ---- main loop over image groups -------------------------------------
            for b in range(NB):
                ng = min(IMG_GRP, B - b * IMG_GRP)
                P = ng * LI
                c0 = b * GP

                # A[P, WF] = sum_c imf_c^T @ sT_c  (+ word mask row)
                a_ps = pa.tile([P, WF], F32, tag="AT")
                for n0, n1 in NCH:
                    for c in range(8):
                        nc.tensor.matmul(
                            a_ps[:, n0:n1], imf[:, c, c0 : c0 + P], sT[:, c, n0:n1],
                            start=(c == 0), stop=False,
                        )
                    nc.tensor.matmul(
                        a_ps[:, n0:n1], orowt[0:1, 0:P], mnegt[0:1, n0:n1],
                        start=False, stop=True,
                    )

                am = work.tile([P, WF], F32, tag="am")
                nc.scalar.copy(am[:], a_ps[:])
                mx = small.tile([P, CAP], F32, tag="mx")
                nc.vector.tensor_reduce(
                    mx[:], a_ps[:].rearrange("p (c w) -> p c w", c=CAP, w=LW),
                    axis=mybir.AxisListType.X, op=mybir.AluOpType.max,
                )
                sub = work.tile([P, WF], F32, tag="sub")
                nc.gpsimd.tensor_tensor(
                    sub[:].rearrange("p (c w) -> p c w", c=CAP, w=LW),
                    am[:].rearrange("p (c w) -> p c w", c=CAP, w=LW),
                    mx[:].unsqueeze(2).broadcast_to([P, CAP, LW]),
                    op=mybir.AluOpType.subtract,
                )
                e = work.tile([P, WF], F32, tag="e")
                nc.scalar.activation(e[:], sub[:], mybir.ActivationFunctionType.Exp)

                z = small.tile([P, CAP], F32, tag="z")
                nc.vector.tensor_reduce(
                    z[:], e[:].rearrange("p (c w) -> p c w", c=CAP, w=LW),
                    axis=mybir.AxisListType.X, op=mybir.AluOpType.add,
                )
                rz = small.tile([P, CAP], F32, tag="rz")
                nc.vector.reciprocal(rz[:], z[:])

                m = work.tile([P, WF], F32, tag="m")
                nc.vector.tensor_tensor(
                    m[:].rearrange("p (c w) -> p c w", c=CAP, w=LW),
                    e[:].rearrange("p (c w) -> p c w", c=CAP, w=LW),
                    rz[:].unsqueeze(2).broadcast_to([P, CAP, LW]),
                    op=mybir.AluOpType.mult,
                )
                # e2 = exp(lam*m - lam) in fp16; the e^-lam scaling cancels in cos
                e2 = work.tile([P, WF], F16, tag="e2")
                nc.scalar.activation(
                    e2[:], m[:], mybir.ActivationFunctionType.Exp,
                    bias=lamneg[0:P, :], scale=LAM,
                )

                f = work.tile([P, WF], F16, tag="f")
                nc.gpsimd.tensor_tensor(f[:], am[:], e2[:], op=mybir.AluOpType.mult)

                t_ps = pa.tile([P, WF], F32, tag="AT")
                for n0, n1 in NCH:
                    nc.tensor.matmul(
                        t_ps[:, n0:n1], gst[0:P, c0 : c0 + P], e2[:, n0:n1],
                        start=True, stop=True,
                    )

                u = work.tile([P, WF], F16, tag="u")
                nc.vector.tensor_tensor(u[:], t_ps[:], e2[:], op=mybir.AluOpType.mult)

                n_ps = pc.tile([ng, WF], F32, tag="cs")
                for n0, n1 in NCH:
                    nc.tensor.matmul(
                        n_ps[:, n0:n1], oblkt[0:P, 0:ng], f[:, n0:n1],
                        start=True, stop=True,
                    )
                w_ps = pc.tile([ng, WF], F32, tag="cs")
                for n0, n1 in NCH:
                    nc.tensor.matmul(
                        w_ps[:, n0:n1], oblkt[0:P, 0:ng], u[:, n0:n1],
                        start=True, stop=True,
                    )

                r0 = b * IMG_GRP
                nb_sb = small.tile([ng, WF], F32, tag="nb_sb")
                wb_sb = small.tile([ng, WF], F32, tag="wb_sb")
                nc.scalar.copy(nb_sb[:], n_ps[:])
                nc.scalar.copy(wb_sb[:], w_ps[:])
                nc.sync.dma_start(nst[r0 : r0 + ng, :], nb_sb[:])
                nc.sync.dma_start(wst[r0 : r0 + ng, :], wb_sb[:])

            # ---- finalize: scores block [128 images, 16 captions] ----------------
            srt = work.tile([128, WF], F32, tag="am")
            nc.scalar.sqrt(srt[:], wst[:])
            q = work.tile([128, WF], F32, tag="e")
            nc.vector.tensor_tensor(q[:], nst[:], wfact[:], op=mybir.AluOpType.mult)
            rsq = work.tile([128, WF], F32, tag="sub")
            nc.vector.reciprocal(rsq[:], srt[:])
            cosq = work.tile([128, WF], F32, tag="m")
            nc.vector.tensor_tensor(cosq[:], q[:], rsq[:], op=mybir.AluOpType.mult)
            sim = small.tile([128, CAP], F32, tag="sim")
            nc.vector.tensor_reduce(
                sim[:], cosq[:].rearrange("p (c w) -> p c w", c=CAP, w=LW),
                axis=mybir.AxisListType.X, op=mybir.AluOpType.add,
            )

            # ---- all-gather the score columns ------------------------------------
            ag_in = dram.tile([128, CAP], F32)
            ag_out = dram.tile([NC, 128, CAP], F32, addr_space="Shared")
            nc.sync.dma_start(ag_in[:], sim[:])
            nc.gpsimd.collective_compute(
                "AllGather",
                mybir.AluOpType.bypass,
                replica_groups=[list(range(NC))],
                ins=[ag_in.opt()],
                outs=[ag_out.opt()],
            )
            s_t = cpool.tile([128, NC, CAP], F32, tag="scores")
            nc.sync.dma_start(s_t[:], ag_out[:].transpose([1, 0, 2]))
            s2d = s_t[:].rearrange("p c w -> p (c w)")
            nc.sync.dma_start(scores_out[:], s2d)

            # ---- margin loss (every core computes it; core 0's is read) ----------
            junk = work.tile([128, 128], F32, tag="am")
            diag = small.tile([128, 1], F32, tag="diag")
            nc.vector.tensor_tensor(junk[:, 0:128], s2d, eyet[:], op=mybir.AluOpType.mult)
            nc.vector.tensor_reduce(
                diag[:], junk[:, 0:128], axis=mybir.AxisListType.X, op=mybir.AluOpType.add
            )
            bias = small.tile([128, 1], F32, tag="bias")
            nc.vector.tensor_scalar(
                bias[:], diag[:], scalar1=-1.0, scalar2=MARGIN,
                op0=mybir.AluOpType.mult, op1=mybir.AluOpType.add,
            )
            # cost_s = relu(S + margin - d_i), diagonal zeroed
            cs = work.tile([128, 128], F32, tag="e")
            nc.scalar.activation(
                cs[:], s2d, mybir.ActivationFunctionType.Relu, bias=bias[:], scale=1.0
            )
            cs2 = work.tile([128, 128], F32, tag="m")
            nc.vector.tensor_tensor(cs2[:], cs[:], neyet[:], op=mybir.AluOpType.mult)
            rmaxs = small.tile([128, 2], F32R, tag="rmaxs")
            nc.vector.tensor_reduce(
                rmaxs[:, 0:1], cs2[:], axis=mybir.AxisListType.X, op=mybir.AluOpType.max
            )
            # transposed scores for cost_im
            st_ps = pc.tile([128, 128], F32, tag="cs")
            nc.tensor.transpose(st_ps[:], s_t[:].rearrange("p c w -> p (c w)"), eyet[:])
            ct = work.tile([128, 128], F32, tag="u")
            nc.scalar.activation(
                ct[:], st_ps[:], mybir.ActivationFunctionType.Relu, bias=bias[:], scale=1.0
            )
            ct2 = work.tile([128, 128], F32, tag="f")
            nc.vector.tensor_tensor(ct2[:], ct[:], neyet[:], op=mybir.AluOpType.mult)
            nc.vector.tensor_reduce(
                rmaxs[:, 1:2], ct2[:], axis=mybir.AxisListType.X, op=mybir.AluOpType.max
            )
            tot_ps = pc.tile([1, 2], F32, tag="cs")
            nc.tensor.matmul(tot_ps[:], oc128t[:], rmaxs[:], start=True, stop=True)
            tot = small.tile([1, 2], F32, tag="tot")
            nc.scalar.copy(tot[:], tot_ps[:])
            nc.sync.dma_start(loss_out[:], tot[:])

    return nc


def _const_arrays():
    """Input-independent constant arrays (built once, device-cached)."""
    bm = np.zeros((GP, GP), dtype=np.float16)
    ob = np.zeros((GP, IMG_GRP), dtype=np.float16)
    for g in range(IMG_GRP):
        bm[g * LI : (g + 1) * LI, g * LI : (g + 1) * LI] = 1.0
        ob[g * LI : (g + 1) * LI, g] = 1.0
    bmask_g = np.ascontiguousarray(np.broadcast_to(bm, (NC, GP, GP))).reshape(
        NC * GP, GP
    )
    oblk_g = np.ascontiguousarray(np.broadcast_to(ob, (NC, GP, IMG_GRP))).reshape(
        NC * GP, IMG_GRP
    )
    orow_g = np.ones((NC * 1, GP), dtype=np.float16)
    o128r_g = np.ones((NC * 1, 128), dtype=np.float32)
    oc128_g = np.ones((NC * 128, 1), dtype=np.float32)
    return {
        "bmask": bmask_g,
        "oblk": oblk_g,
        "orow": orow_g,
        "o128r": o128r_g,
        "oc128": oc128_g,
    }


def _prep_imh(im):
    # imh global [NC*8, 128, SH]: core c gets imT8[:, :, c*SH:(c+1)*SH]
    im = np.ascontiguousarray(im, dtype=np.float32)
    return (
        im.reshape(NC, SH, 8, 128).transpose(0, 2, 3, 1).astype(np.float16)
    ).reshape(NC * 8, 128, SH)


def _prep_sTh(s):
    # sTh global [NC*8, 128, WF]: core c gets its own 16 captions, transposed
    s = np.ascontiguousarray(s, dtype=np.float32)
    return (
        s.reshape(NC, WF, 8, 128).transpose(0, 2, 3, 1).astype(np.float16)
    ).reshape(NC * 8, 128, WF)


def _prep_small(s, s_l):
    s_l = np.asarray(s_l).astype(np.int64)
    wmask = np.arange(LW)[None, :] < s_l[:, None]                  # [B, LW]
    capn = np.sqrt(np.einsum("bwd,bwd->bw", s.astype(np.float32), s.astype(np.float32)))
    lens = s_l.astype(np.float32)[:, None]
    mneg_g = ((~wmask) * np.float32(MASKNEG)).astype(np.float16).reshape(NC, WF)
    wfac_g = (
        (wmask / (np.maximum(capn, EPS) * lens)).astype(np.float32).reshape(NC, WF)
    )
    return {"mneg": mneg_g, "wfacr": wfac_g}


def _get_state():
    if "state" in _CACHE:
        return _CACHE["state"]
    _install_patches()

    import jax
    from jax.sharding import Mesh, PartitionSpec, NamedSharding
    import warnings

    with warnings.catch_warnings():
        warnings.simplefilter("ignore")
        from jax.experimental.shard_map import shard_map

    from concourse.bass2jax import (
        _bass_exec_p,
        install_neuronx_cc_hook,
        partition_id_tensor,
    )

    install_neuronx_cc_hook()
    nc = _build_program()

    partition_name = nc.partition_id_tensor.name if nc.partition_id_tensor else None
    in_names, out_names, out_avals, zero_shapes = [], [], [], []
    for alloc in nc.m.functions[0].allocations:
        if not isinstance(alloc, mybir.MemoryLocationSet):
            continue
        name = alloc.memorylocations[0].name
        if alloc.kind == "ExternalInput":
            if name != partition_name:
                in_names.append(name)
        elif alloc.kind == "ExternalOutput":
            shape = tuple(alloc.tensor_shape)
            dtype = mybir.dt.np(alloc.dtype)
            out_names.append(name)
            out_avals.append(jax.core.ShapedArray(shape, dtype))
            zero_shapes.append((shape, dtype))
    n_params = len(in_names)
    n_outs = len(out_avals)
    in_names_all = in_names + out_names
    if partition_name is not None:
        in_names_all = in_names_all + [partition_name]
    donate = tuple(range(n_params, n_params + n_outs))

    def _body(*args):
        operands = list(args)
        if partition_name is not None:
            operands.append(partition_id_tensor())
        outs = _bass_exec_p.bind(
            *operands,
            out_avals=tuple(out_avals),
            in_names=tuple(in_names_all),
            out_names=tuple(out_names),
            lowering_input_output_aliases=(),
            sim_require_finite=True,
            sim_require_nnan=True,
            nc=nc,
        )
        return tuple(outs)

    devices = jax.devices()[:NC]
    assert len(devices) == NC
    mesh = Mesh(np.asarray(devices), ("core",))
    spec = NamedSharding(mesh, PartitionSpec("core"))
    sharded = jax.jit(
        shard_map(
            _body,
            mesh=mesh,
            in_specs=(PartitionSpec("core"),) * (n_params + n_outs),
            out_specs=(PartitionSpec("core"),) * n_outs,
            check_rep=False,
        ),
        donate_argnums=donate,
        keep_unused=True,
    )

    # donation buffers built on-device (no wire traffic)
    import jax.numpy as jnp

    def _mkzeros():
        return tuple(
            jnp.zeros((NC * sh[0], *sh[1:]), dt) for sh, dt in zero_shapes
        )

    zeros_fn = jax.jit(_mkzeros, out_shardings=(spec,) * n_outs)

    # input-independent constants: transferred once, reused across calls
    const_np = _const_arrays()
    const_dev = {n: jax.device_put(a, spec) for n, a in const_np.items()}

    state = {
        "nc": nc,
        "sharded": sharded,
        "spec": spec,
        "in_names": in_names,
        "out_names": out_names,
        "zeros_fn": zeros_fn,
        "const_dev": const_dev,
        "loss_idx": out_names.index("loss_out"),
        "scores_idx": out_names.index("scores_out"),
    }
    _CACHE["state"] = state
    return state


def run(im, s, s_l, fetch_scores=False, trace=False):
    """Returns (loss_scalar, scores[128,128] or None)."""
    import jax

    st = _get_state()
    spec = st["spec"]
    # kick off on-device zero-buffer creation (async, no wire traffic)
    zeros = st["zeros_fn"]()
    # pipeline host prep with the tunnel transfers: put each big array as
    # soon as it is built
    dev = dict(st["const_dev"])
    dev["imh"] = jax.device_put(_prep_imh(im), spec)
    dev["sTh"] = jax.device_put(_prep_sTh(s), spec)
    for n, a in _prep_small(s, s_l).items():
        dev[n] = jax.device_put(a, spec)
    outs = st["sharded"](*[dev[n] for n in st["in_names"]], *zeros)
    loss_arr = outs[st["loss_idx"]]
    try:
        l0 = np.asarray(loss_arr.addressable_shards[0].data)
    except Exception:
        l0 = np.asarray(loss_arr)[0:1]
    loss = np.float32(l0[0, 0] + l0[0, 1])
    scores = None
    if fetch_scores:
        scores = np.asarray(outs[st["scores_idx"]])[0:128]
    return loss, scores


def kernel(im, s, s_l):
    loss, _ = run(im, s, s_l)
    return np.array(loss, dtype=np.float32)


# revision 17
# speedup vs baseline: 1.3856x; 1.1949x over previous
"""Trainium2 Bass kernel for nn_ContrastiveLoss (stacked cross-attention t2i).

Strategy (8 NeuronCores, caption-sharded; wire-traffic optimized):
  - The axon tunnel to the devices is the bottleneck (~46 MB/s), so inputs
    are minimized: im is shipped SHARDED (1/8 per core, fp16) and
    all-gathered on-device over NeuronLink; s is shipped per-core (its own
    16 captions, fp16); Gram matrices, eye/noteye and the broadcast wfac
    are built on-device.
  - Each core owns 16 of the 128 captions and all 128 images.  Per batch of
    3 images x 16 captions, compute A = im @ s^T on the PE (fp16 in, f32
    accum), the two softmaxes (word softmax normalized; region softmax's
    normalizer cancels inside the cosine, so only exp(9*a1 - 9) is needed
    -- the e^-9 scaling keeps everything fp16-safe), the cosine
    numerator/denominator via PE column sums, staged into [128, 800] tiles.
  - One finalization pass turns staged tiles into the [128, 16] score block.
  - AllGather score blocks -> every core holds scores [128, 128]; the hinge
    margin loss (max violation) is computed on-device; host reads core 0's
    scalar.
  - The jitted shard_map dispatch is built once and cached; per-call work is
    host prep (~0.1 s), ~24 MB of input transfer, and the device run.

Math note: with E2 = exp(lam * a1 - lam) (unnormalized region attention),
  cos = (sum_r E2*A) / (cap_n * sqrt(E2^T G E2)) exactly, because both the
region-softmax normalizer and the e^-lam scaling cancel between numerator
and |weighted context|.
"""

import numpy as np

import concourse.bass as bass
import concourse.tile as tile
from concourse import mybir
from concourse.vector_clock import ScopedClock

# ---------------------------------------------------------------------------
# Workaround for this toolchain: walrus rejects instructions carrying more
# than one semaphore wait.  Split extra waits onto standalone EventSemaphore
# instructions (the same thing wait_ge emits) just before the offender.
# ---------------------------------------------------------------------------
_PATCHED = False


def _install_patches():
    global _PATCHED
    if _PATCHED:
        return
    _PATCHED = True

    def _drain_and_barrier(self, tick_clock, wait_clock):
        nc = self.nc
        drain_inst = nc.sync.drain()
        wait_clock.add_sem_waits(
            drain_inst.ins, ScopedClock({None: tick_clock.global_clock})
        )
        waits = list(drain_inst.ins.sync_info.on_wait)
        if len(waits) > 1:
            drain_inst.ins.sync_info.on_wait = waits[:1]
            for w in waits[1:]:
                extra = nc.sync.drain()
                extra.ins.sync_info = mybir.SyncInfo(on_wait=[w], on_update=[])
        nc.all_engine_barrier()
        popped = nc._tile_sem_poison_stack.pop()
        assert popped is self._sem_poison
        nc.clear_and_free_semaphores(list(self.sems.allocated().values()))
        nc.all_engine_barrier()

    tile.TileContext._drain_and_barrier = _drain_and_barrier

    import concourse.bass_utils as bass_utils
    import concourse.bass2jax as bass2jax
    import orjson

    _orig_compile = bass_utils.compile_bir_kernel

    def _split_waits_in_bir(bir_json: bytes) -> bytes:
        m = orjson.loads(bir_json)
        for fn in m.get("functions", []):
            for blk in fn.get("blocks", []):
                insts = blk.get("instructions", [])
                new_insts = []
                for ins in insts:
                    si = ins.get("sync_info")
                    waits = (si or {}).get("on_wait") or []
                    if len(waits) > 1:
                        for k, w in enumerate(waits[:-1]):
                            new_insts.append(
                                {
                                    "name": f"{ins['name']}_wsplit{k}",
                                    "opcode": "EventSemaphore",
                                    "engine": ins["engine"],
                                    "ins": [],
                                    "outs": [],
                                    "debug": ins.get("debug"),
                                    "sync_info": {"on_update": [], "on_wait": [w]},
                                }
                            )
                        si["on_wait"] = waits[-1:]
                    new_insts.append(ins)
                blk["instructions"] = new_insts
        return orjson.dumps(m)

    def _patched_compile(bir_json, tmpdir, neff_name="file.neff"):
        return _orig_compile(_split_waits_in_bir(bir_json), tmpdir, neff_name)

    bass_utils.compile_bir_kernel = _patched_compile
    bass2jax.compile_bir_kernel = _patched_compile


# ---------------------------------------------------------------------------
# Problem constants (hardcoded per the task contract).
# ---------------------------------------------------------------------------
B = 128           # images == captions
LI = 36           # image regions
LW = 50           # padded caption words
D = 1024          # feature dim
NC = 8            # cores
CAP = B // NC     # captions per core (16)
WF = CAP * LW     # free width of the batched tiles (800)
IMG_GRP = 3       # images per batch
GP = IMG_GRP * LI  # partitions per full batch (108)
NB = (B + IMG_GRP - 1) // IMG_GRP  # 43 batches (42x3 + 1x2)
SH = B * LI // NC  # im columns per shard (576)
LAM = 9.0
MARGIN = 0.2
EPS = 1e-8
MASKNEG = -30000.0

F32 = mybir.dt.float32
F32R = mybir.dt.float32r
F16 = mybir.dt.float16
I32 = mybir.dt.int32

_CACHE = {}


def _build_program(lks):
    """lks: per-slot caption word counts (len CAP, same on every core).

    The wire tensor sTh ships only sum(lks) words per (core, slot); on-device
    DMAs scatter each slot back into the fixed [CAP, LW] padded layout.
    """
    SP = int(sum(lks))
    nc = bass.Bass("TRN2", target_bir_lowering=False, debug=False, num_devices=NC)

    # Inputs (per-core contents differ only for sTh / mneg / wfacr).
    imh = nc.dram_tensor("imh", [8, 128, SH], F16, kind="ExternalInput")
    sTh = nc.dram_tensor("sTh", [8, 128, SP], F16, kind="ExternalInput")
    mneg = nc.dram_tensor("mneg", [1, WF], F16, kind="ExternalInput")
    wfacr = nc.dram_tensor("wfacr", [1, WF], F32R, kind="ExternalInput")
    bmask = nc.dram_tensor("bmask", [GP, GP], F16, kind="ExternalInput")
    oblk = nc.dram_tensor("oblk", [GP, IMG_GRP], F16, kind="ExternalInput")
    orow = nc.dram_tensor("orow", [1, GP], F16, kind="ExternalInput")
    o128r = nc.dram_tensor("o128r", [1, 128], F32R, kind="ExternalInput")
    oc128 = nc.dram_tensor("oc128", [128, 1], F32R, kind="ExternalInput")

    loss_out = nc.dram_tensor("loss_out", [1, 2], F32, kind="ExternalOutput")
    scores_out = nc.dram_tensor("scores_out", [128, 128], F32, kind="ExternalOutput")

    with tile.TileContext(nc) as tc:
        with (
            tc.tile_pool(name="const", bufs=1) as cpool,
            tc.tile_pool(name="work", bufs=2) as work,
            tc.tile_pool(name="small", bufs=2) as small,
            tc.tile_pool(name="stage", bufs=1) as stage,
            tc.tile_pool(name="pa", bufs=2, space="PSUM") as pa,
            tc.tile_pool(name="pc", bufs=2, space="PSUM") as pc,
            tc.tile_pool(name="dram", bufs=1, space="DRAM") as dram,
        ):
            NCH = [(0, 512), (512, WF)]

            # ---- persistent consts ------------------------------------------------
            # unpack slot-packed captions into the fixed [CAP, LW] layout;
            # words in [lk, LW) stay whatever memset left (masked by mneg)
            sT = cpool.tile([128, 8, WF], F16, tag="sT")
            nc.gpsimd.memset(sT[:], 0.0)
            off = 0
            for k in range(CAP):
                lk = int(lks[k])
                nc.sync.dma_start(
                    sT[:, :, k * LW : k * LW + lk],
                    sTh[:, :, off : off + lk].transpose([1, 0, 2]),
                )
                off += lk
            mnegt = cpool.tile([1, WF], F16, tag="mn")
            nc.sync.dma_start(mnegt[:], mneg[:])
            wfacrt = cpool.tile([1, WF], F32R, tag="wfr")
            nc.sync.dma_start(wfacrt[:], wfacr[:])
            bmaskt = cpool.tile([GP, GP], F16, tag="bm")
            nc.sync.dma_start(bmaskt[:], bmask[:])
            oblkt = cpool.tile([GP, IMG_GRP], F16, tag="ob")
            nc.sync.dma_start(oblkt[:], oblk[:])
            orowt = cpool.tile([1, GP], F16, tag="or")
            nc.sync.dma_start(orowt[:], orow[:])
            o128rt = cpool.tile([1, 128], F32R, tag="o128r")
            nc.sync.dma_start(o128rt[:], o128r[:])
            oc128t = cpool.tile([128, 1], F32R, tag="oc128")
            nc.sync.dma_start(oc128t[:], oc128[:])

            # eye / noteye generated on-device (saves wire bytes)
            iit = cpool.tile([128, 128], I32, tag="ii")
            nc.gpsimd.iota(iit[:], pattern=[[1, 128]], base=0, channel_multiplier=-1)
            eyet = cpool.tile([128, 128], F32, tag="eye")
            nc.vector.tensor_scalar(
                eyet[:], iit[:], scalar1=0, scalar2=None,
                op0=mybir.AluOpType.is_equal,
            )
            neyet = cpool.tile([128, 128], F32, tag="neye")
            nc.vector.tensor_scalar(
                neyet[:], iit[:], scalar1=0, scalar2=None,
                op0=mybir.AluOpType.not_equal,
            )

            lamneg = cpool.tile([128, 1], F32, tag="lamneg")
            nc.vector.memset(lamneg[:], -LAM)

            # wfac broadcast to all 128 partitions via K=1 matmul
            wf_ps = pc.tile([128, WF], F32, tag="cs")
            for n0, n1 in NCH:
                nc.tensor.matmul(
                    wf_ps[:, n0:n1], o128rt[0:1, :], wfacrt[0:1, n0:n1],
                    start=True, stop=True,
                )
            wfact = cpool.tile([128, WF], F32, tag="wf")
            nc.scalar.copy(wfact[:], wf_ps[:])

            # ---- all-gather im shards over NeuronLink -----------------------------
            agi = dram.tile([8, 128, SH], F16)
            nc.sync.dma_start(agi[:], imh[:])
            ago = dram.tile([NC, 8, 128, SH], F16, addr_space="Shared")
            nc.gpsimd.collective_compute(
                "AllGather",
                mybir.AluOpType.bypass,
                replica_groups=[list(range(NC))],
                ins=[agi.opt()],
                outs=[ago.opt()],
            )
            imf = cpool.tile([128, 8, B * LI], F16, tag="imf")
            for sh in range(NC):
                nc.sync.dma_start(
                    imf[:, :, sh * SH : (sh + 1) * SH],
                    ago[sh].transpose([1, 0, 2]),
                )

            # ---- staging tiles ----------------------------------------------------
            nst = stage.tile([128, WF], F32, tag="nst")
            wst = stage.tile([128, WF], F32, tag="wst")
            gst = stage.tile([GP, NB * GP], F16, tag="gst")

            # ---- G pass: all 43 block-diagonal Gram blocks ------------------------
            for b in range(NB):
                ng = min(IMG_GRP, B - b * IMG_GRP)
                P = ng * LI
                c0 = b * GP
                g_ps = pa.tile([GP, WF], F32, tag="AT")
                for c in range(8):
                    nc.tensor.matmul(
                        g_ps[0:P, 0:P], imf[:, c, c0 : c0 + P], imf[:, c, c0 : c0 + P],
                        start=(c == 0), stop=(c == 7),
                    )
                nc.vector.tensor_tensor(
                    gst[0:P, c0 : c0 + P], g_ps[0:P, 0:P], bmaskt[0:P, 0:P],
                    op=mybir.AluOpType.mult,
                )

            # ---- main loop over image groups -------------------------------------
            for b in range(NB):
                ng = min(IMG_GRP, B - b * IMG_GRP)
                P = ng * LI
                c0 = b * GP

                # A[P, WF] = sum_c imf_c^T @ sT_c  (+ word mask row)
                a_ps = pa.tile([P, WF], F32, tag="AT")
                for n0, n1 in NCH:
                    for c in range(8):
                        nc.tensor.matmul(
                            a_ps[:, n0:n1], imf[:, c, c0 : c0 + P], sT[:, c, n0:n1],
                            start=(c == 0), stop=False,
                        )
                    nc.tensor.matmul(
                        a_ps[:, n0:n1], orowt[0:1, 0:P], mnegt[0:1, n0:n1],
                        start=False, stop=True,
                    )

                am = work.tile([P, WF], F32, tag="am")
                nc.scalar.copy(am[:], a_ps[:])
                mx = small.tile([P, CAP], F32, tag="mx")
                nc.vector.tensor_reduce(
                    mx[:], a_ps[:].rearrange("p (c w) -> p c w", c=CAP, w=LW),
                    axis=mybir.AxisListType.X, op=mybir.AluOpType.max,
                )
                sub = work.tile([P, WF], F32, tag="sub")
                nc.gpsimd.tensor_tensor(
                    sub[:].rearrange("p (c w) -> p c w", c=CAP, w=LW),
                    am[:].rearrange("p (c w) -> p c w", c=CAP, w=LW),
                    mx[:].unsqueeze(2).broadcast_to([P, CAP, LW]),
                    op=mybir.AluOpType.subtract,
                )
                e = work.tile([P, WF], F32, tag="e")
                nc.scalar.activation(e[:], sub[:], mybir.ActivationFunctionType.Exp)

                z = small.tile([P, CAP], F32, tag="z")
                nc.vector.tensor_reduce(
                    z[:], e[:].rearrange("p (c w) -> p c w", c=CAP, w=LW),
                    axis=mybir.AxisListType.X, op=mybir.AluOpType.add,
                )
                rz = small.tile([P, CAP], F32, tag="rz")
                nc.vector.reciprocal(rz[:], z[:])

                m = work.tile([P, WF], F32, tag="m")
                nc.vector.tensor_tensor(
                    m[:].rearrange("p (c w) -> p c w", c=CAP, w=LW),
                    e[:].rearrange("p (c w) -> p c w", c=CAP, w=LW),
                    rz[:].unsqueeze(2).broadcast_to([P, CAP, LW]),
                    op=mybir.AluOpType.mult,
                )
                # e2 = exp(lam*m - lam) in fp16; the e^-lam scaling cancels in cos
                e2 = work.tile([P, WF], F16, tag="e2")
                nc.scalar.activation(
                    e2[:], m[:], mybir.ActivationFunctionType.Exp,
                    bias=lamneg[0:P, :], scale=LAM,
                )

                f = work.tile([P, WF], F16, tag="f")
                nc.gpsimd.tensor_tensor(f[:], am[:], e2[:], op=mybir.AluOpType.mult)

                t_ps = pa.tile([P, WF], F32, tag="AT")
                for n0, n1 in NCH:
                    nc.tensor.matmul(
                        t_ps[:, n0:n1], gst[0:P, c0 : c0 + P], e2[:, n0:n1],
                        start=True, stop=True,
                    )

                u = work.tile([P, WF], F16, tag="u")
                nc.vector.tensor_tensor(u[:], t_ps[:], e2[:], op=mybir.AluOpType.mult)

                n_ps = pc.tile([ng, WF], F32, tag="cs")
                for n0, n1 in NCH:
                    nc.tensor.matmul(
                        n_ps[:, n0:n1], oblkt[0:P, 0:ng], f[:, n0:n1],
                        start=True, stop=True,
                    )
                w_ps = pc.tile([ng, WF], F32, tag="cs")
                for n0, n1 in NCH:
                    nc.tensor.matmul(
                        w_ps[:, n0:n1], oblkt[0:P, 0:ng], u[:, n0:n1],
                        start=True, stop=True,
                    )

                r0 = b * IMG_GRP
                nb_sb = small.tile([ng, WF], F32, tag="nb_sb")
                wb_sb = small.tile([ng, WF], F32, tag="wb_sb")
                nc.scalar.copy(nb_sb[:], n_ps[:])
                nc.scalar.copy(wb_sb[:], w_ps[:])
                nc.sync.dma_start(nst[r0 : r0 + ng, :], nb_sb[:])
                nc.sync.dma_start(wst[r0 : r0 + ng, :], wb_sb[:])

            # ---- finalize: scores block [128 images, 16 captions] ----------------
            srt = work.tile([128, WF], F32, tag="am")
            nc.scalar.sqrt(srt[:], wst[:])
            q = work.tile([128, WF], F32, tag="e")
            nc.vector.tensor_tensor(q[:], nst[:], wfact[:], op=mybir.AluOpType.mult)
            rsq = work.tile([128, WF], F32, tag="sub")
            nc.vector.reciprocal(rsq[:], srt[:])
            cosq = work.tile([128, WF], F32, tag="m")
            nc.vector.tensor_tensor(cosq[:], q[:], rsq[:], op=mybir.AluOpType.mult)
            sim = small.tile([128, CAP], F32, tag="sim")
            nc.vector.tensor_reduce(
                sim[:], cosq[:].rearrange("p (c w) -> p c w", c=CAP, w=LW),
                axis=mybir.AxisListType.X, op=mybir.AluOpType.add,
            )

            # ---- all-gather the score columns ------------------------------------
            ag_in = dram.tile([128, CAP], F32)
            ag_out = dram.tile([NC, 128, CAP], F32, addr_space="Shared")
            nc.sync.dma_start(ag_in[:], sim[:])
            nc.gpsimd.collective_compute(
                "AllGather",
                mybir.AluOpType.bypass,
                replica_groups=[list(range(NC))],
                ins=[ag_in.opt()],
                outs=[ag_out.opt()],
            )
            s_t = cpool.tile([128, NC, CAP], F32, tag="scores")
            nc.sync.dma_start(s_t[:], ag_out[:].transpose([1, 0, 2]))
            s2d = s_t[:].rearrange("p c w -> p (c w)")
            nc.sync.dma_start(scores_out[:], s2d)

            # ---- margin loss (every core computes it; core 0's is read) ----------
            junk = work.tile([128, 128], F32, tag="am")
            diag = small.tile([128, 1], F32, tag="diag")
            nc.vector.tensor_tensor(junk[:, 0:128], s2d, eyet[:], op=mybir.AluOpType.mult)
            nc.vector.tensor_reduce(
                diag[:], junk[:, 0:128], axis=mybir.AxisListType.X, op=mybir.AluOpType.add
            )
            bias = small.tile([128, 1], F32, tag="bias")
            nc.vector.tensor_scalar(
                bias[:], diag[:], scalar1=-1.0, scalar2=MARGIN,
                op0=mybir.AluOpType.mult, op1=mybir.AluOpType.add,
            )
            # cost_s = relu(S + margin - d_i), diagonal zeroed
            cs = work.tile([128, 128], F32, tag="e")
            nc.scalar.activation(
                cs[:], s2d, mybir.ActivationFunctionType.Relu, bias=bias[:], scale=1.0
            )
            cs2 = work.tile([128, 128], F32, tag="m")
            nc.vector.tensor_tensor(cs2[:], cs[:], neyet[:], op=mybir.AluOpType.mult)
            rmaxs = small.tile([128, 2], F32R, tag="rmaxs")
            nc.vector.tensor_reduce(
                rmaxs[:, 0:1], cs2[:], axis=mybir.AxisListType.X, op=mybir.AluOpType.max
            )
            # transposed scores for cost_im
            st_ps = pc.tile([128, 128], F32, tag="cs")
            nc.tensor.transpose(st_ps[:], s_t[:].rearrange("p c w -> p (c w)"), eyet[:])
            ct = work.tile([128, 128], F32, tag="u")
            nc.scalar.activation(
                ct[:], st_ps[:], mybir.ActivationFunctionType.Relu, bias=bias[:], scale=1.0
            )
            ct2 = work.tile([128, 128], F32, tag="f")
            nc.vector.tensor_tensor(ct2[:], ct[:], neyet[:], op=mybir.AluOpType.mult)
            nc.vector.tensor_reduce(
                rmaxs[:, 1:2], ct2[:], axis=mybir.AxisListType.X, op=mybir.AluOpType.max
            )
            tot_ps = pc.tile([1, 2], F32, tag="cs")
            nc.tensor.matmul(tot_ps[:], oc128t[:], rmaxs[:], start=True, stop=True)
            tot = small.tile([1, 2], F32, tag="tot")
            nc.scalar.copy(tot[:], tot_ps[:])
            nc.sync.dma_start(loss_out[:], tot[:])

    return nc


def _const_arrays():
    """Input-independent constant arrays (built once, device-cached)."""
    bm = np.zeros((GP, GP), dtype=np.float16)
    ob = np.zeros((GP, IMG_GRP), dtype=np.float16)
    for g in range(IMG_GRP):
        bm[g * LI : (g + 1) * LI, g * LI : (g + 1) * LI] = 1.0
        ob[g * LI : (g + 1) * LI, g] = 1.0
    bmask_g = np.ascontiguousarray(np.broadcast_to(bm, (NC, GP, GP))).reshape(
        NC * GP, GP
    )
    oblk_g = np.ascontiguousarray(np.broadcast_to(ob, (NC, GP, IMG_GRP))).reshape(
        NC * GP, IMG_GRP
    )
    orow_g = np.ones((NC * 1, GP), dtype=np.float16)
    o128r_g = np.ones((NC * 1, 128), dtype=np.float32)
    oc128_g = np.ones((NC * 128, 1), dtype=np.float32)
    return {
        "bmask": bmask_g,
        "oblk": oblk_g,
        "orow": orow_g,
        "o128r": o128r_g,
        "oc128": oc128_g,
    }


def _perm_and_lks(s_l):
    """Sort captions by length desc, round-robin over cores.

    Device column (core c, slot k) holds caption order[8k + c]; slot k's
    baked width is the max length in its group (= first, sorted desc).
    perm[c*CAP + k] = order[8k + c] is the caption (and image) permutation.
    """
    s_l = np.asarray(s_l).astype(np.int64)
    order = np.argsort(-s_l, kind="stable")
    lks = tuple(int(s_l[order[NC * k]]) for k in range(CAP))
    perm = order.reshape(CAP, NC).T.reshape(B)
    return perm, order, lks


def _prep_imh(im, perm):
    # imh global [NC*8, 128, SH]: core c gets imT8[:, :, c*SH:(c+1)*SH]
    # (image rows permuted to match the caption permutation)
    im = np.ascontiguousarray(im, dtype=np.float32)[perm]
    return (
        im.reshape(NC, SH, 8, 128).transpose(0, 2, 3, 1).astype(np.float16)
    ).reshape(NC * 8, 128, SH)


def _prep_sTh(s, order, lks):
    # slot-packed captions: global [NC*8, 128, SP]
    SP = int(sum(lks))
    s = np.ascontiguousarray(s, dtype=np.float32)
    out = np.empty((NC, 8, 128, SP), dtype=np.float16)
    off = 0
    for k in range(CAP):
        lk = int(lks[k])
        ids = order[NC * k : NC * (k + 1)]          # core c -> ids[c]
        blk = s[ids][:, :lk, :]                     # [NC, lk, D]
        out[:, :, :, off : off + lk] = (
            blk.transpose(0, 2, 1).astype(np.float16).reshape(NC, 8, 128, lk)
        )
        off += lk
    return out.reshape(NC * 8, 128, SP)


def _prep_small(s, s_l, perm):
    s_l = np.asarray(s_l).astype(np.int64)[perm]
    s = np.ascontiguousarray(s, dtype=np.float32)
    wmask = np.arange(LW)[None, :] < s_l[:, None]                  # [B, LW]
    capn = np.sqrt(np.einsum("bwd,bwd->bw", s, s))[perm]
    lens = s_l.astype(np.float32)[:, None]
    mneg_g = ((~wmask) * np.float32(MASKNEG)).astype(np.float16).reshape(NC, WF)
    wfac_g = (
        (wmask / (np.maximum(capn, EPS) * lens)).astype(np.float32).reshape(NC, WF)
    )
    return {"mneg": mneg_g, "wfacr": wfac_g}


def _get_state(lks):
    key = ("state", tuple(lks))
    if key in _CACHE:
        return _CACHE[key]
    _install_patches()

    import jax
    from jax.sharding import Mesh, PartitionSpec, NamedSharding
    import warnings

    with warnings.catch_warnings():
        warnings.simplefilter("ignore")
        from jax.experimental.shard_map import shard_map

    from concourse.bass2jax import (
        _bass_exec_p,
        install_neuronx_cc_hook,
        partition_id_tensor,
    )

    install_neuronx_cc_hook()
    nc = _build_program(lks)

    partition_name = nc.partition_id_tensor.name if nc.partition_id_tensor else None
    in_names, out_names, out_avals, zero_shapes = [], [], [], []
    for alloc in nc.m.functions[0].allocations:
        if not isinstance(alloc, mybir.MemoryLocationSet):
            continue
        name = alloc.memorylocations[0].name
        if alloc.kind == "ExternalInput":
            if name != partition_name:
                in_names.append(name)
        elif alloc.kind == "ExternalOutput":
            shape = tuple(alloc.tensor_shape)
            dtype = mybir.dt.np(alloc.dtype)
            out_names.append(name)
            out_avals.append(jax.core.ShapedArray(shape, dtype))
            zero_shapes.append((shape, dtype))
    n_params = len(in_names)
    n_outs = len(out_avals)
    in_names_all = in_names + out_names
    if partition_name is not None:
        in_names_all = in_names_all + [partition_name]
    donate = tuple(range(n_params, n_params + n_outs))

    def _body(*args):
        operands = list(args)
        if partition_name is not None:
            operands.append(partition_id_tensor())
        outs = _bass_exec_p.bind(
            *operands,
            out_avals=tuple(out_avals),
            in_names=tuple(in_names_all),
            out_names=tuple(out_names),
            lowering_input_output_aliases=(),
            sim_require_finite=True,
            sim_require_nnan=True,
            nc=nc,
        )
        return tuple(outs)

    devices = jax.devices()[:NC]
    assert len(devices) == NC
    mesh = Mesh(np.asarray(devices), ("core",))
    spec = NamedSharding(mesh, PartitionSpec("core"))
    sharded = jax.jit(
        shard_map(
            _body,
            mesh=mesh,
            in_specs=(PartitionSpec("core"),) * (n_params + n_outs),
            out_specs=(PartitionSpec("core"),) * n_outs,
            check_rep=False,
        ),
        donate_argnums=donate,
        keep_unused=True,
    )

    # donation buffers built on-device (no wire traffic)
    import jax.numpy as jnp

    def _mkzeros():
        return tuple(
            jnp.zeros((NC * sh[0], *sh[1:]), dt) for sh, dt in zero_shapes
        )

    zeros_fn = jax.jit(_mkzeros, out_shardings=(spec,) * n_outs)

    # input-independent constants: transferred once, reused across calls
    const_np = _const_arrays()
    const_dev = {n: jax.device_put(a, spec) for n, a in const_np.items()}

    state = {
        "nc": nc,
        "sharded": sharded,
        "spec": spec,
        "in_names": in_names,
        "out_names": out_names,
        "zeros_fn": zeros_fn,
        "const_dev": const_dev,
        "loss_idx": out_names.index("loss_out"),
        "scores_idx": out_names.index("scores_out"),
    }
    _CACHE[key] = state
    return state


def run(im, s, s_l, fetch_scores=False, trace=False):
    """Returns (loss_scalar, scores[128,128] or None)."""
    import jax

    perm, order, lks = _perm_and_lks(s_l)
    st = _get_state(lks)
    spec = st["spec"]
    # kick off on-device zero-buffer creation (async, no wire traffic)
    zeros = st["zeros_fn"]()
    # pipeline host prep with the tunnel transfers: put each big array as
    # soon as it is built
    dev = dict(st["const_dev"])
    dev["imh"] = jax.device_put(_prep_imh(im, perm), spec)
    dev["sTh"] = jax.device_put(_prep_sTh(s, order, lks), spec)
    for n, a in _prep_small(s, s_l, perm).items():
        dev[n] = jax.device_put(a, spec)
    outs = st["sharded"](*[dev[n] for n in st["in_names"]], *zeros)
    loss_arr = outs[st["loss_idx"]]
    try:
        l0 = np.asarray(loss_arr.addressable_shards[0].data)
    except Exception:
        l0 = np.asarray(loss_arr)[0:1]
    loss = np.float32(l0[0, 0] + l0[0, 1])
    scores = None
    if fetch_scores:
        # device scores are in permuted (image, caption) order; undo it
        sd = np.asarray(outs[st["scores_idx"]])[0:128]
        scores = np.empty_like(sd)
        scores[np.ix_(perm, perm)] = sd
    return loss, scores


def kernel(im, s, s_l):
    loss, _ = run(im, s, s_l)
    return np.array(loss, dtype=np.float32)


# revision 18
# speedup vs baseline: 2.1390x; 1.5437x over previous
"""Trainium2 Bass kernel for nn_ContrastiveLoss (stacked cross-attention t2i).

Strategy (8 NeuronCores, caption-sharded; wire-traffic optimized):
  - The axon tunnel to the devices is the bottleneck (~46 MB/s), so inputs
    are minimized: im is shipped SHARDED (1/8 per core, fp16) and
    all-gathered on-device over NeuronLink; s is shipped per-core (its own
    16 captions, fp16); Gram matrices, eye/noteye and the broadcast wfac
    are built on-device.
  - Each core owns 16 of the 128 captions and all 128 images.  Per batch of
    3 images x 16 captions, compute A = im @ s^T on the PE (fp16 in, f32
    accum), the two softmaxes (word softmax normalized; region softmax's
    normalizer cancels inside the cosine, so only exp(9*a1 - 9) is needed
    -- the e^-9 scaling keeps everything fp16-safe), the cosine
    numerator/denominator via PE column sums, staged into [128, 800] tiles.
  - One finalization pass turns staged tiles into the [128, 16] score block.
  - AllGather score blocks -> every core holds scores [128, 128]; the hinge
    margin loss (max violation) is computed on-device; host reads core 0's
    scalar.
  - The jitted shard_map dispatch is built once and cached; per-call work is
    host prep (~0.1 s), ~24 MB of input transfer, and the device run.

Math note: with E2 = exp(lam * a1 - lam) (unnormalized region attention),
  cos = (sum_r E2*A) / (cap_n * sqrt(E2^T G E2)) exactly, because both the
region-softmax normalizer and the e^-lam scaling cancel between numerator
and |weighted context|.
"""

import numpy as np

import concourse.bass as bass
import concourse.tile as tile
from concourse import mybir
from concourse.vector_clock import ScopedClock

# ---------------------------------------------------------------------------
# Workaround for this toolchain: walrus rejects instructions carrying more
# than one semaphore wait.  Split extra waits onto standalone EventSemaphore
# instructions (the same thing wait_ge emits) just before the offender.
# ---------------------------------------------------------------------------
_PATCHED = False


def _install_patches():
    global _PATCHED
    if _PATCHED:
        return
    _PATCHED = True

    def _drain_and_barrier(self, tick_clock, wait_clock):
        nc = self.nc
        drain_inst = nc.sync.drain()
        wait_clock.add_sem_waits(
            drain_inst.ins, ScopedClock({None: tick_clock.global_clock})
        )
        waits = list(drain_inst.ins.sync_info.on_wait)
        if len(waits) > 1:
            drain_inst.ins.sync_info.on_wait = waits[:1]
            for w in waits[1:]:
                extra = nc.sync.drain()
                extra.ins.sync_info = mybir.SyncInfo(on_wait=[w], on_update=[])
        nc.all_engine_barrier()
        popped = nc._tile_sem_poison_stack.pop()
        assert popped is self._sem_poison
        nc.clear_and_free_semaphores(list(self.sems.allocated().values()))
        nc.all_engine_barrier()

    tile.TileContext._drain_and_barrier = _drain_and_barrier

    import concourse.bass_utils as bass_utils
    import concourse.bass2jax as bass2jax
    import orjson

    _orig_compile = bass_utils.compile_bir_kernel

    def _split_waits_in_bir(bir_json: bytes) -> bytes:
        m = orjson.loads(bir_json)
        for fn in m.get("functions", []):
            for blk in fn.get("blocks", []):
                insts = blk.get("instructions", [])
                new_insts = []
                for ins in insts:
                    si = ins.get("sync_info")
                    waits = (si or {}).get("on_wait") or []
                    if len(waits) > 1:
                        for k, w in enumerate(waits[:-1]):
                            new_insts.append(
                                {
                                    "name": f"{ins['name']}_wsplit{k}",
                                    "opcode": "EventSemaphore",
                                    "engine": ins["engine"],
                                    "ins": [],
                                    "outs": [],
                                    "debug": ins.get("debug"),
                                    "sync_info": {"on_update": [], "on_wait": [w]},
                                }
                            )
                        si["on_wait"] = waits[-1:]
                    new_insts.append(ins)
                blk["instructions"] = new_insts
        return orjson.dumps(m)

    def _patched_compile(bir_json, tmpdir, neff_name="file.neff"):
        return _orig_compile(_split_waits_in_bir(bir_json), tmpdir, neff_name)

    bass_utils.compile_bir_kernel = _patched_compile
    bass2jax.compile_bir_kernel = _patched_compile


# ---------------------------------------------------------------------------
# Problem constants (hardcoded per the task contract).
# ---------------------------------------------------------------------------
B = 128           # images == captions
LI = 36           # image regions
LW = 50           # padded caption words
D = 1024          # feature dim
NC = 8            # cores
CAP = B // NC     # captions per core (16)
WF = CAP * LW     # free width of the batched tiles (800)
IMG_GRP = 3       # images per batch
GP = IMG_GRP * LI  # partitions per full batch (108)
NB = (B + IMG_GRP - 1) // IMG_GRP  # 43 batches (42x3 + 1x2)
SH = B * LI // NC  # im columns per shard (576)
LAM = 9.0
MARGIN = 0.2
EPS = 1e-8
MASKNEG = -240.0  # fp8e4m3 max finite; gap still underflows exp

F32 = mybir.dt.float32
F32R = mybir.dt.float32r
F16 = mybir.dt.float16
F8 = mybir.dt.float8e4
NPF8 = mybir.dt.np(F8)
I32 = mybir.dt.int32

_CACHE = {}


def _build_program(lks):
    """lks: per-slot caption word counts (len CAP, same on every core).

    The wire tensor sTh ships only sum(lks) words per (core, slot); on-device
    DMAs scatter each slot back into the fixed [CAP, LW] padded layout.
    """
    SP = int(sum(lks))
    nc = bass.Bass("TRN2", target_bir_lowering=False, debug=False, num_devices=NC)

    # Inputs (per-core contents differ only for sTh / mneg / wfacr).
    imh = nc.dram_tensor("imh", [8, 128, SH], F8, kind="ExternalInput")
    sTh = nc.dram_tensor("sTh", [8, 128, SP], F8, kind="ExternalInput")
    mneg = nc.dram_tensor("mneg", [1, WF], F8, kind="ExternalInput")
    wfacr = nc.dram_tensor("wfacr", [1, WF], F32R, kind="ExternalInput")
    bmask = nc.dram_tensor("bmask", [GP, GP], F16, kind="ExternalInput")
    oblk = nc.dram_tensor("oblk", [GP, IMG_GRP], F16, kind="ExternalInput")
    orow = nc.dram_tensor("orow", [1, GP], F8, kind="ExternalInput")
    o128r = nc.dram_tensor("o128r", [1, 128], F32R, kind="ExternalInput")
    oc128 = nc.dram_tensor("oc128", [128, 1], F32R, kind="ExternalInput")

    loss_out = nc.dram_tensor("loss_out", [1, 2], F32, kind="ExternalOutput")
    scores_out = nc.dram_tensor("scores_out", [128, 128], F32, kind="ExternalOutput")

    with tile.TileContext(nc) as tc:
        with (
            tc.tile_pool(name="const", bufs=1) as cpool,
            tc.tile_pool(name="work", bufs=2) as work,
            tc.tile_pool(name="small", bufs=2) as small,
            tc.tile_pool(name="stage", bufs=1) as stage,
            tc.tile_pool(name="pa", bufs=2, space="PSUM") as pa,
            tc.tile_pool(name="pc", bufs=2, space="PSUM") as pc,
            tc.tile_pool(name="dram", bufs=1, space="DRAM") as dram,
        ):
            NCH = [(0, 512), (512, WF)]

            # ---- persistent consts ------------------------------------------------
            # unpack slot-packed captions into the fixed [CAP, LW] layout;
            # words in [lk, LW) stay whatever memset left (masked by mneg)
            sT = cpool.tile([128, 8, WF], F8, tag="sT")
            nc.gpsimd.memset(sT[:], 0.0)
            off = 0
            for k in range(CAP):
                lk = int(lks[k])
                nc.sync.dma_start(
                    sT[:, :, k * LW : k * LW + lk],
                    sTh[:, :, off : off + lk].transpose([1, 0, 2]),
                )
                off += lk
            mnegt = cpool.tile([1, WF], F8, tag="mn")
            nc.sync.dma_start(mnegt[:], mneg[:])
            wfacrt = cpool.tile([1, WF], F32R, tag="wfr")
            nc.sync.dma_start(wfacrt[:], wfacr[:])
            bmaskt = cpool.tile([GP, GP], F16, tag="bm")
            nc.sync.dma_start(bmaskt[:], bmask[:])
            oblkt = cpool.tile([GP, IMG_GRP], F16, tag="ob")
            nc.sync.dma_start(oblkt[:], oblk[:])
            orowt = cpool.tile([1, GP], F8, tag="or")
            nc.sync.dma_start(orowt[:], orow[:])
            o128rt = cpool.tile([1, 128], F32R, tag="o128r")
            nc.sync.dma_start(o128rt[:], o128r[:])
            oc128t = cpool.tile([128, 1], F32R, tag="oc128")
            nc.sync.dma_start(oc128t[:], oc128[:])

            # eye / noteye generated on-device (saves wire bytes)
            iit = cpool.tile([128, 128], I32, tag="ii")
            nc.gpsimd.iota(iit[:], pattern=[[1, 128]], base=0, channel_multiplier=-1)
            eyet = cpool.tile([128, 128], F32, tag="eye")
            nc.vector.tensor_scalar(
                eyet[:], iit[:], scalar1=0, scalar2=None,
                op0=mybir.AluOpType.is_equal,
            )
            neyet = cpool.tile([128, 128], F32, tag="neye")
            nc.vector.tensor_scalar(
                neyet[:], iit[:], scalar1=0, scalar2=None,
                op0=mybir.AluOpType.not_equal,
            )

            lamneg = cpool.tile([128, 1], F32, tag="lamneg")
            nc.vector.memset(lamneg[:], -LAM)

            # wfac broadcast to all 128 partitions via K=1 matmul
            wf_ps = pc.tile([128, WF], F32, tag="cs")
            for n0, n1 in NCH:
                nc.tensor.matmul(
                    wf_ps[:, n0:n1], o128rt[0:1, :], wfacrt[0:1, n0:n1],
                    start=True, stop=True,
                )
            wfact = cpool.tile([128, WF], F32, tag="wf")
            nc.scalar.copy(wfact[:], wf_ps[:])

            # ---- all-gather im shards over NeuronLink -----------------------------
            agi = dram.tile([8, 128, SH], F8)
            nc.sync.dma_start(agi[:], imh[:])
            ago = dram.tile([NC, 8, 128, SH], F8, addr_space="Shared")
            nc.gpsimd.collective_compute(
                "AllGather",
                mybir.AluOpType.bypass,
                replica_groups=[list(range(NC))],
                ins=[agi.opt()],
                outs=[ago.opt()],
            )
            imf = cpool.tile([128, 8, B * LI], F8, tag="imf")
            for sh in range(NC):
                nc.sync.dma_start(
                    imf[:, :, sh * SH : (sh + 1) * SH],
                    ago[sh].transpose([1, 0, 2]),
                )

            # ---- staging tiles ----------------------------------------------------
            nst = stage.tile([128, WF], F32, tag="nst")
            wst = stage.tile([128, WF], F32, tag="wst")
            gst = stage.tile([GP, NB * GP], F16, tag="gst")

            # ---- G pass: all 43 block-diagonal Gram blocks ------------------------
            for b in range(NB):
                ng = min(IMG_GRP, B - b * IMG_GRP)
                P = ng * LI
                c0 = b * GP
                g_ps = pa.tile([GP, WF], F32, tag="AT")
                for c in range(8):
                    nc.tensor.matmul(
                        g_ps[0:P, 0:P], imf[:, c, c0 : c0 + P], imf[:, c, c0 : c0 + P],
                        start=(c == 0), stop=(c == 7),
                    )
                nc.vector.tensor_tensor(
                    gst[0:P, c0 : c0 + P], g_ps[0:P, 0:P], bmaskt[0:P, 0:P],
                    op=mybir.AluOpType.mult,
                )

            # ---- main loop over image groups -------------------------------------
            for b in range(NB):
                ng = min(IMG_GRP, B - b * IMG_GRP)
                P = ng * LI
                c0 = b * GP

                # A[P, WF] = sum_c imf_c^T @ sT_c  (+ word mask row)
                a_ps = pa.tile([P, WF], F32, tag="AT")
                for n0, n1 in NCH:
                    for c in range(8):
                        nc.tensor.matmul(
                            a_ps[:, n0:n1], imf[:, c, c0 : c0 + P], sT[:, c, n0:n1],
                            start=(c == 0), stop=False,
                        )
                    nc.tensor.matmul(
                        a_ps[:, n0:n1], orowt[0:1, 0:P], mnegt[0:1, n0:n1],
                        start=False, stop=True,
                    )

                am = work.tile([P, WF], F32, tag="am")
                nc.scalar.copy(am[:], a_ps[:])
                mx = small.tile([P, CAP], F32, tag="mx")
                nc.vector.tensor_reduce(
                    mx[:], a_ps[:].rearrange("p (c w) -> p c w", c=CAP, w=LW),
                    axis=mybir.AxisListType.X, op=mybir.AluOpType.max,
                )
                sub = work.tile([P, WF], F32, tag="sub")
                nc.gpsimd.tensor_tensor(
                    sub[:].rearrange("p (c w) -> p c w", c=CAP, w=LW),
                    am[:].rearrange("p (c w) -> p c w", c=CAP, w=LW),
                    mx[:].unsqueeze(2).broadcast_to([P, CAP, LW]),
                    op=mybir.AluOpType.subtract,
                )
                e = work.tile([P, WF], F32, tag="e")
                nc.scalar.activation(e[:], sub[:], mybir.ActivationFunctionType.Exp)

                z = small.tile([P, CAP], F32, tag="z")
                nc.vector.tensor_reduce(
                    z[:], e[:].rearrange("p (c w) -> p c w", c=CAP, w=LW),
                    axis=mybir.AxisListType.X, op=mybir.AluOpType.add,
                )
                rz = small.tile([P, CAP], F32, tag="rz")
                nc.vector.reciprocal(rz[:], z[:])

                m = work.tile([P, WF], F32, tag="m")
                nc.vector.tensor_tensor(
                    m[:].rearrange("p (c w) -> p c w", c=CAP, w=LW),
                    e[:].rearrange("p (c w) -> p c w", c=CAP, w=LW),
                    rz[:].unsqueeze(2).broadcast_to([P, CAP, LW]),
                    op=mybir.AluOpType.mult,
                )
                # e2 = exp(lam*m - lam) in fp16; the e^-lam scaling cancels in cos
                e2 = work.tile([P, WF], F16, tag="e2")
                nc.scalar.activation(
                    e2[:], m[:], mybir.ActivationFunctionType.Exp,
                    bias=lamneg[0:P, :], scale=LAM,
                )

                f = work.tile([P, WF], F16, tag="f")
                nc.gpsimd.tensor_tensor(f[:], am[:], e2[:], op=mybir.AluOpType.mult)

                t_ps = pa.tile([P, WF], F32, tag="AT")
                for n0, n1 in NCH:
                    nc.tensor.matmul(
                        t_ps[:, n0:n1], gst[0:P, c0 : c0 + P], e2[:, n0:n1],
                        start=True, stop=True,
                    )

                u = work.tile([P, WF], F16, tag="u")
                nc.vector.tensor_tensor(u[:], t_ps[:], e2[:], op=mybir.AluOpType.mult)

                n_ps = pc.tile([ng, WF], F32, tag="cs")
                for n0, n1 in NCH:
                    nc.tensor.matmul(
                        n_ps[:, n0:n1], oblkt[0:P, 0:ng], f[:, n0:n1],
                        start=True, stop=True,
                    )
                w_ps = pc.tile([ng, WF], F32, tag="cs")
                for n0, n1 in NCH:
                    nc.tensor.matmul(
                        w_ps[:, n0:n1], oblkt[0:P, 0:ng], u[:, n0:n1],
                        start=True, stop=True,
                    )

                r0 = b * IMG_GRP
                nb_sb = small.tile([ng, WF], F32, tag="nb_sb")
                wb_sb = small.tile([ng, WF], F32, tag="wb_sb")
                nc.scalar.copy(nb_sb[:], n_ps[:])
                nc.scalar.copy(wb_sb[:], w_ps[:])
                nc.sync.dma_start(nst[r0 : r0 + ng, :], nb_sb[:])
                nc.sync.dma_start(wst[r0 : r0 + ng, :], wb_sb[:])

            # ---- finalize: scores block [128 images, 16 captions] ----------------
            srt = work.tile([128, WF], F32, tag="am")
            nc.scalar.sqrt(srt[:], wst[:])
            q = work.tile([128, WF], F32, tag="e")
            nc.vector.tensor_tensor(q[:], nst[:], wfact[:], op=mybir.AluOpType.mult)
            rsq = work.tile([128, WF], F32, tag="sub")
            nc.vector.reciprocal(rsq[:], srt[:])
            cosq = work.tile([128, WF], F32, tag="m")
            nc.vector.tensor_tensor(cosq[:], q[:], rsq[:], op=mybir.AluOpType.mult)
            sim = small.tile([128, CAP], F32, tag="sim")
            nc.vector.tensor_reduce(
                sim[:], cosq[:].rearrange("p (c w) -> p c w", c=CAP, w=LW),
                axis=mybir.AxisListType.X, op=mybir.AluOpType.add,
            )

            # ---- all-gather the score columns ------------------------------------
            ag_in = dram.tile([128, CAP], F32)
            ag_out = dram.tile([NC, 128, CAP], F32, addr_space="Shared")
            nc.sync.dma_start(ag_in[:], sim[:])
            nc.gpsimd.collective_compute(
                "AllGather",
                mybir.AluOpType.bypass,
                replica_groups=[list(range(NC))],
                ins=[ag_in.opt()],
                outs=[ag_out.opt()],
            )
            s_t = cpool.tile([128, NC, CAP], F32, tag="scores")
            nc.sync.dma_start(s_t[:], ag_out[:].transpose([1, 0, 2]))
            s2d = s_t[:].rearrange("p c w -> p (c w)")
            nc.sync.dma_start(scores_out[:], s2d)

            # ---- margin loss (every core computes it; core 0's is read) ----------
            junk = work.tile([128, 128], F32, tag="am")
            diag = small.tile([128, 1], F32, tag="diag")
            nc.vector.tensor_tensor(junk[:, 0:128], s2d, eyet[:], op=mybir.AluOpType.mult)
            nc.vector.tensor_reduce(
                diag[:], junk[:, 0:128], axis=mybir.AxisListType.X, op=mybir.AluOpType.add
            )
            bias = small.tile([128, 1], F32, tag="bias")
            nc.vector.tensor_scalar(
                bias[:], diag[:], scalar1=-1.0, scalar2=MARGIN,
                op0=mybir.AluOpType.mult, op1=mybir.AluOpType.add,
            )
            # cost_s = relu(S + margin - d_i), diagonal zeroed
            cs = work.tile([128, 128], F32, tag="e")
            nc.scalar.activation(
                cs[:], s2d, mybir.ActivationFunctionType.Relu, bias=bias[:], scale=1.0
            )
            cs2 = work.tile([128, 128], F32, tag="m")
            nc.vector.tensor_tensor(cs2[:], cs[:], neyet[:], op=mybir.AluOpType.mult)
            rmaxs = small.tile([128, 2], F32R, tag="rmaxs")
            nc.vector.tensor_reduce(
                rmaxs[:, 0:1], cs2[:], axis=mybir.AxisListType.X, op=mybir.AluOpType.max
            )
            # transposed scores for cost_im
            st_ps = pc.tile([128, 128], F32, tag="cs")
            nc.tensor.transpose(st_ps[:], s_t[:].rearrange("p c w -> p (c w)"), eyet[:])
            ct = work.tile([128, 128], F32, tag="u")
            nc.scalar.activation(
                ct[:], st_ps[:], mybir.ActivationFunctionType.Relu, bias=bias[:], scale=1.0
            )
            ct2 = work.tile([128, 128], F32, tag="f")
            nc.vector.tensor_tensor(ct2[:], ct[:], neyet[:], op=mybir.AluOpType.mult)
            nc.vector.tensor_reduce(
                rmaxs[:, 1:2], ct2[:], axis=mybir.AxisListType.X, op=mybir.AluOpType.max
            )
            tot_ps = pc.tile([1, 2], F32, tag="cs")
            nc.tensor.matmul(tot_ps[:], oc128t[:], rmaxs[:], start=True, stop=True)
            tot = small.tile([1, 2], F32, tag="tot")
            nc.scalar.copy(tot[:], tot_ps[:])
            nc.sync.dma_start(loss_out[:], tot[:])

    return nc


def _const_arrays():
    """Input-independent constant arrays (built once, device-cached)."""
    bm = np.zeros((GP, GP), dtype=np.float16)
    ob = np.zeros((GP, IMG_GRP), dtype=np.float16)
    for g in range(IMG_GRP):
        bm[g * LI : (g + 1) * LI, g * LI : (g + 1) * LI] = 1.0
        ob[g * LI : (g + 1) * LI, g] = 1.0
    bmask_g = np.ascontiguousarray(np.broadcast_to(bm, (NC, GP, GP))).reshape(
        NC * GP, GP
    )
    oblk_g = np.ascontiguousarray(np.broadcast_to(ob, (NC, GP, IMG_GRP))).reshape(
        NC * GP, IMG_GRP
    )
    orow_g = np.ones((NC * 1, GP), dtype=NPF8)
    o128r_g = np.ones((NC * 1, 128), dtype=np.float32)
    oc128_g = np.ones((NC * 128, 1), dtype=np.float32)
    return {
        "bmask": bmask_g,
        "oblk": oblk_g,
        "orow": orow_g,
        "o128r": o128r_g,
        "oc128": oc128_g,
    }


def _perm_and_lks(s_l):
    """Sort captions by length desc, round-robin over cores.

    Device column (core c, slot k) holds caption order[8k + c]; slot k's
    baked width is the max length in its group (= first, sorted desc).
    perm[c*CAP + k] = order[8k + c] is the caption (and image) permutation.
    """
    s_l = np.asarray(s_l).astype(np.int64)
    order = np.argsort(-s_l, kind="stable")
    lks = tuple(int(s_l[order[NC * k]]) for k in range(CAP))
    perm = order.reshape(CAP, NC).T.reshape(B)
    return perm, order, lks


def _prep_imh(im, perm):
    # imh global [NC*8, 128, SH]: core c gets imT8[:, :, c*SH:(c+1)*SH]
    # (image rows permuted to match the caption permutation)
    im = np.ascontiguousarray(im, dtype=np.float32)[perm]
    return (
        im.reshape(NC, SH, 8, 128).transpose(0, 2, 3, 1).astype(NPF8)
    ).reshape(NC * 8, 128, SH)


def _prep_sTh(s, order, lks):
    # slot-packed captions: global [NC*8, 128, SP]
    SP = int(sum(lks))
    s = np.ascontiguousarray(s, dtype=np.float32)
    out = np.empty((NC, 8, 128, SP), dtype=NPF8)
    off = 0
    for k in range(CAP):
        lk = int(lks[k])
        ids = order[NC * k : NC * (k + 1)]          # core c -> ids[c]
        blk = s[ids][:, :lk, :]                     # [NC, lk, D]
        out[:, :, :, off : off + lk] = (
            blk.transpose(0, 2, 1).astype(NPF8).reshape(NC, 8, 128, lk)
        )
        off += lk
    return out.reshape(NC * 8, 128, SP)


def _prep_small(s, s_l, perm):
    s_l = np.asarray(s_l).astype(np.int64)[perm]
    s = np.ascontiguousarray(s, dtype=np.float32)
    wmask = np.arange(LW)[None, :] < s_l[:, None]                  # [B, LW]
    capn = np.sqrt(np.einsum("bwd,bwd->bw", s, s))[perm]
    lens = s_l.astype(np.float32)[:, None]
    mneg_g = ((~wmask) * np.float32(MASKNEG)).astype(NPF8).reshape(NC, WF)
    wfac_g = (
        (wmask / (np.maximum(capn, EPS) * lens)).astype(np.float32).reshape(NC, WF)
    )
    return {"mneg": mneg_g, "wfacr": wfac_g}


def _get_state(lks):
    key = ("state", tuple(lks))
    if key in _CACHE:
        return _CACHE[key]
    _install_patches()

    import jax
    from jax.sharding import Mesh, PartitionSpec, NamedSharding
    import warnings

    with warnings.catch_warnings():
        warnings.simplefilter("ignore")
        from jax.experimental.shard_map import shard_map

    from concourse.bass2jax import (
        _bass_exec_p,
        install_neuronx_cc_hook,
        partition_id_tensor,
    )

    install_neuronx_cc_hook()
    nc = _build_program(lks)

    partition_name = nc.partition_id_tensor.name if nc.partition_id_tensor else None
    in_names, out_names, out_avals, zero_shapes = [], [], [], []
    for alloc in nc.m.functions[0].allocations:
        if not isinstance(alloc, mybir.MemoryLocationSet):
            continue
        name = alloc.memorylocations[0].name
        if alloc.kind == "ExternalInput":
            if name != partition_name:
                in_names.append(name)
        elif alloc.kind == "ExternalOutput":
            shape = tuple(alloc.tensor_shape)
            dtype = mybir.dt.np(alloc.dtype)
            out_names.append(name)
            out_avals.append(jax.core.ShapedArray(shape, dtype))
            zero_shapes.append((shape, dtype))
    n_params = len(in_names)
    n_outs = len(out_avals)
    in_names_all = in_names + out_names
    if partition_name is not None:
        in_names_all = in_names_all + [partition_name]
    donate = tuple(range(n_params, n_params + n_outs))

    def _body(*args):
        operands = list(args)
        if partition_name is not None:
            operands.append(partition_id_tensor())
        outs = _bass_exec_p.bind(
            *operands,
            out_avals=tuple(out_avals),
            in_names=tuple(in_names_all),
            out_names=tuple(out_names),
            lowering_input_output_aliases=(),
            sim_require_finite=True,
            sim_require_nnan=True,
            nc=nc,
        )
        return tuple(outs)

    devices = jax.devices()[:NC]
    assert len(devices) == NC
    mesh = Mesh(np.asarray(devices), ("core",))
    spec = NamedSharding(mesh, PartitionSpec("core"))
    sharded = jax.jit(
        shard_map(
            _body,
            mesh=mesh,
            in_specs=(PartitionSpec("core"),) * (n_params + n_outs),
            out_specs=(PartitionSpec("core"),) * n_outs,
            check_rep=False,
        ),
        donate_argnums=donate,
        keep_unused=True,
    )

    # donation buffers built on-device (no wire traffic)
    import jax.numpy as jnp

    def _mkzeros():
        return tuple(
            jnp.zeros((NC * sh[0], *sh[1:]), dt) for sh, dt in zero_shapes
        )

    zeros_fn = jax.jit(_mkzeros, out_shardings=(spec,) * n_outs)

    # input-independent constants: transferred once, reused across calls
    const_np = _const_arrays()
    const_dev = {n: jax.device_put(a, spec) for n, a in const_np.items()}

    state = {
        "nc": nc,
        "sharded": sharded,
        "spec": spec,
        "in_names": in_names,
        "out_names": out_names,
        "zeros_fn": zeros_fn,
        "const_dev": const_dev,
        "loss_idx": out_names.index("loss_out"),
        "scores_idx": out_names.index("scores_out"),
    }
    _CACHE[key] = state
    return state


def run(im, s, s_l, fetch_scores=False, trace=False):
    """Returns (loss_scalar, scores[128,128] or None)."""
    import jax

    perm, order, lks = _perm_and_lks(s_l)
    st = _get_state(lks)
    spec = st["spec"]
    # kick off on-device zero-buffer creation (async, no wire traffic)
    zeros = st["zeros_fn"]()
    # pipeline host prep with the tunnel transfers: put each big array as
    # soon as it is built
    dev = dict(st["const_dev"])
    dev["imh"] = jax.device_put(_prep_imh(im, perm), spec)
    dev["sTh"] = jax.device_put(_prep_sTh(s, order, lks), spec)
    for n, a in _prep_small(s, s_l, perm).items():
        dev[n] = jax.device_put(a, spec)
    outs = st["sharded"](*[dev[n] for n in st["in_names"]], *zeros)
    loss_arr = outs[st["loss_idx"]]
    try:
        l0 = np.asarray(loss_arr.addressable_shards[0].data)
    except Exception:
        l0 = np.asarray(loss_arr)[0:1]
    loss = np.float32(l0[0, 0] + l0[0, 1])
    scores = None
    if fetch_scores:
        # device scores are in permuted (image, caption) order; undo it
        sd = np.asarray(outs[st["scores_idx"]])[0:128]
        scores = np.empty_like(sd)
        scores[np.ix_(perm, perm)] = sd
    return loss, scores


def kernel(im, s, s_l):
    loss, _ = run(im, s, s_l)
    return np.array(loss, dtype=np.float32)
